# revision 12
# baseline (speedup 1.0000x reference)
"""Trainium2 Bass kernel for nn_CILRSModel (moe_routing).

Strategy:
  - Host-side MoE routing: rows are bucketed by `command` (6 branches) and
    distributed evenly over 8 cores. Each core gets a fixed [6 x CAP] row
    layout so the SPMD kernel statically knows which branch weights apply
    to which batch tile (no on-device routing control flow).
  - Host-side transpose: the embedding (+ speed scalar) is shipped
    feature-major ([513, rows_per_core]) so every matmul operand already
    has the contraction dim on SBUF partitions - zero on-device transposes.
  - On device, everything is feature-major: x [640, N] tiles flow through
    speed-MLP, speed head and the (single, routed) branch head on the PE;
    PSUM is evicted with fused bias+ReLU (tensor_scalar / activation)
    split across the Vector and Scalar engines.
  - Outputs come back feature-major as out4 = [4, rows] (3 control rows +
    1 speed row) and are scattered back to the original row order on host.
"""

import os
import sys

import numpy as np

_TRN_REPO = "/opt/trn_rl_repo"
if _TRN_REPO not in sys.path:
    sys.path.insert(0, _TRN_REPO)

# Problem constants (hardcoded per harness contract)
B = 65536
D_EMB = 512
D_LAT = 128
H = 256
NBRANCH = 6
D_IN = D_EMB + D_LAT  # 640
NCORES = 8
CAP = 1536            # per-core per-branch row capacity (actual max ~1389)
BPC = NBRANCH * CAP   # 9216 rows per core
NT = 512              # batch-tile size (matmul free dim)
NTILES = BPC // NT    # 18
TPB = CAP // NT       # tiles per branch slot = 3

_cache = {}


# --------------------------------------------------------------------------
# Device kernel
# --------------------------------------------------------------------------

def _build_nc():
    if "nc" in _cache:
        return _cache["nc"]

    import concourse.mybir as mybir
    import concourse.tile as tile
    from concourse import bacc
    from concourse.bass import ts

    f32 = mybir.dt.float32
    # matmul operand dtype (non-f32 -> LDW split; bf16 streams 2 cols/cycle)
    f16 = getattr(mybir.dt, os.environ.get("KERNEL_DT", "float16"))
    AF = mybir.ActivationFunctionType
    ALU = mybir.AluOpType

    nc = bacc.Bacc("TRN2", target_bir_lowering=False, debug=False,
                   num_devices=NCORES)

    def din(name, shape, dt=f16):
        return nc.dram_tensor(name, list(shape), dt, kind="ExternalInput")[:]

    xt = din("xt", [D_EMB + 1, BPC])           # rows 0..511 emb.T, row 512 speed
    wsi1 = din("wsi1", [1, H])
    bsi1 = din("bsi1", [128, 2], f32)
    wsi2 = din("wsi2", [128, 2, D_LAT])
    bsi2 = din("bsi2", [128, 1], f32)
    wso1 = din("wso1", [128, 5, H])
    bso1 = din("bso1", [128, 2], f32)
    wso2 = din("wso2", [128, 2, H])
    bso2 = din("bso2", [128, 2], f32)
    wso3 = din("wso3", [128, 2, 1])
    bso3 = din("bso3", [1, 1], f32)
    wb1 = din("wb1", [128, 5, NBRANCH, H])
    bb1 = din("bb1", [128, 2, NBRANCH], f32)
    wb2 = din("wb2", [128, 2, NBRANCH, H])
    bb2 = din("bb2", [128, 2, NBRANCH], f32)
    wb3 = din("wb3", [128, 2, NBRANCH, 3])
    bb3 = din("bb3", [3, NBRANCH], f32)
    out4 = nc.dram_tensor("out4", [4, BPC], f32, kind="ExternalOutput")[:]

    with tile.TileContext(nc) as tc:
        with (
            tc.tile_pool(name="wpool", bufs=1) as wpool,
            tc.tile_pool(name="xpool", bufs=3) as xpool,
            tc.tile_pool(name="hpool", bufs=2) as hpool,
            tc.tile_pool(name="opool", bufs=1) as opool,
            tc.tile_pool(name="pmm", bufs=3, space="PSUM") as pmm,
            tc.tile_pool(name="pm1", bufs=2, space="PSUM") as pm1,
        ):
            def loadw(ap, shape, tag, dt=f16):
                t = wpool.tile(list(shape), dt, tag=tag)
                nc.sync.dma_start(out=t[:], in_=ap)
                return t

            wsi1_s = loadw(wsi1, [1, H], "wsi1")
            bsi1_s = loadw(bsi1, [128, 2], "bsi1", f32)
            wsi2_s = loadw(wsi2, [128, 2, D_LAT], "wsi2")
            bsi2_s = loadw(bsi2, [128, 1], "bsi2", f32)
            wso1_s = loadw(wso1, [128, 5, H], "wso1")
            bso1_s = loadw(bso1, [128, 2], "bso1", f32)
            wso2_s = loadw(wso2, [128, 2, H], "wso2")
            bso2_s = loadw(bso2, [128, 2], "bso2", f32)
            wso3_s = loadw(wso3, [128, 2, 1], "wso3")
            bso3_s = loadw(bso3, [1, 1], "bso3", f32)
            wb1_s = loadw(wb1, [128, 5, NBRANCH, H], "wb1")
            bb1_s = loadw(bb1, [128, 2, NBRANCH], "bb1", f32)
            wb2_s = loadw(wb2, [128, 2, NBRANCH, H], "wb2")
            bb2_s = loadw(bb2, [128, 2, NBRANCH], "bb2", f32)
            wb3_s = loadw(wb3, [128, 2, NBRANCH, 3], "wb3")
            bb3_s = loadw(bb3, [3, NBRANCH], "bb3", f32)

            ctl_s = opool.tile([3, BPC], f32, tag="octl")
            spd_s = opool.tile([1, BPC], f32, tag="ospd")

            xt_emb = xt[0:D_EMB, :].rearrange("(o p) b -> p o b", p=128)

            def evict_relu(dst, src, bias_ap, on_act):
                if on_act:
                    nc.scalar.activation(dst, src, AF.Relu, bias=bias_ap)
                else:
                    nc.vector.tensor_scalar(dst, src, bias_ap, 0.0, ALU.add, ALU.max)

            for t in range(NTILES):
                k = t // TPB
                cols = ts(t, NT)

                x_s = xpool.tile([128, 5, NT], f16, tag="x")
                nc.sync.dma_start(out=x_s[:, 0:4, :], in_=xt_emb[:, :, cols])
                nc.sync.dma_start(out=x_s[0:1, 4, :], in_=xt[D_EMB:D_EMB + 1, cols])

                # ---- speed-input MLP, layer 1 (K=1 matmuls on the speed row)
                p_h = pmm.tile([128, 2, NT], f32, tag="pmm")
                nc.tensor.matmul(p_h[:, 0, :], wsi1_s[0:1, 0:128], x_s[0:1, 4, :],
                                 start=True, stop=True)
                nc.tensor.matmul(p_h[:, 1, :], wsi1_s[0:1, 128:256], x_s[0:1, 4, :],
                                 start=True, stop=True)
                hsp = hpool.tile([128, 2, NT], f16, tag="hsp")
                evict_relu(hsp[:, 0, :], p_h[:, 0, :], bsi1_s[:, 0:1], True)
                evict_relu(hsp[:, 1, :], p_h[:, 1, :], bsi1_s[:, 1:2], False)

                # ---- speed head layer 1, emb subtiles only (o=0..3), keeps PE
                # busy while the speed latent is computed
                p1 = pmm.tile([128, 2, NT], f32, tag="pmm")
                for j in range(2):
                    for o in range(4):
                        nc.tensor.matmul(p1[:, j, :], wso1_s[:, o, ts(j, 128)],
                                         x_s[:, o, :], start=(o == 0), stop=False)

                # ---- speed-input MLP, layer 2 -> speed latent into x_s[:,4,:]
                p_sp = pm1.tile([128, NT], f32, tag="pm1")
                nc.tensor.matmul(p_sp[:], wsi2_s[:, 0, :], hsp[:, 0, :],
                                 start=True, stop=False)
                nc.tensor.matmul(p_sp[:], wsi2_s[:, 1, :], hsp[:, 1, :],
                                 start=False, stop=True)

                # ---- branch layer 1, emb subtiles only
                q1 = pmm.tile([128, 2, NT], f32, tag="pmm")
                for j in range(2):
                    for o in range(4):
                        nc.tensor.matmul(q1[:, j, :], wb1_s[:, o, k, ts(j, 128)],
                                         x_s[:, o, :], start=(o == 0), stop=False)

                # speed latent eviction (bias add, no relu)
                nc.vector.tensor_scalar(x_s[:, 4, :], p_sp[:], bsi2_s[:, 0:1],
                                        None, ALU.add)

                # ---- finish layer-1 accumulations with the latent subtile o=4
                for j in range(2):
                    nc.tensor.matmul(p1[:, j, :], wso1_s[:, 4, ts(j, 128)],
                                     x_s[:, 4, :], start=False, stop=True)
                for j in range(2):
                    nc.tensor.matmul(q1[:, j, :], wb1_s[:, 4, k, ts(j, 128)],
                                     x_s[:, 4, :], start=False, stop=True)

                h1 = hpool.tile([128, 2, NT], f16, tag="h1")
                evict_relu(h1[:, 0, :], p1[:, 0, :], bso1_s[:, 0:1], True)
                evict_relu(h1[:, 1, :], p1[:, 1, :], bso1_s[:, 1:2], False)

                # ---- speed head layer 2
                p2 = pmm.tile([128, 2, NT], f32, tag="pmm")
                for j in range(2):
                    for o in range(2):
                        nc.tensor.matmul(p2[:, j, :], wso2_s[:, o, ts(j, 128)],
                                         h1[:, o, :], start=(o == 0), stop=(o == 1))

                g1 = hpool.tile([128, 2, NT], f16, tag="g1")
                evict_relu(g1[:, 0, :], q1[:, 0, :], bb1_s[:, 0, k:k + 1], True)
                evict_relu(g1[:, 1, :], q1[:, 1, :], bb1_s[:, 1, k:k + 1], False)

                # ---- branch layer 2
                q2 = pmm.tile([128, 2, NT], f32, tag="pmm")
                for j in range(2):
                    for o in range(2):
                        nc.tensor.matmul(q2[:, j, :], wb2_s[:, o, k, ts(j, 128)],
                                         g1[:, o, :], start=(o == 0), stop=(o == 1))

                h2 = hpool.tile([128, 2, NT], f16, tag="h2")
                evict_relu(h2[:, 0, :], p2[:, 0, :], bso2_s[:, 0:1], True)
                evict_relu(h2[:, 1, :], p2[:, 1, :], bso2_s[:, 1:2], False)

                # ---- speed head output (M=1)
                p_o = pm1.tile([1, NT], f32, tag="pm1")
                nc.tensor.matmul(p_o[:], wso3_s[:, 0, :], h2[:, 0, :],
                                 start=True, stop=False)
                nc.tensor.matmul(p_o[:], wso3_s[:, 1, :], h2[:, 1, :],
                                 start=False, stop=True)

                g2 = hpool.tile([128, 2, NT], f16, tag="g2")
                evict_relu(g2[:, 0, :], q2[:, 0, :], bb2_s[:, 0, k:k + 1], True)
                evict_relu(g2[:, 1, :], q2[:, 1, :], bb2_s[:, 1, k:k + 1], False)

                nc.vector.tensor_scalar(spd_s[:, cols], p_o[:], bso3_s[0:1, 0:1],
                                        None, ALU.add)

                # ---- branch output (M=3) + sigmoid
                p_c = pm1.tile([3, NT], f32, tag="pm1")
                nc.tensor.matmul(p_c[:], wb3_s[:, 0, k, :], g2[:, 0, :],
                                 start=True, stop=False)
                nc.tensor.matmul(p_c[:], wb3_s[:, 1, k, :], g2[:, 1, :],
                                 start=False, stop=True)
                nc.scalar.activation(ctl_s[:, cols], p_c[:], AF.Sigmoid,
                                     bias=bb3_s[:, k:k + 1])

            nc.sync.dma_start(out=out4[0:3, :], in_=ctl_s[:])
            nc.sync.dma_start(out=out4[3:4, :], in_=spd_s[:])

    nc.compile()
    _cache["nc"] = nc
    return nc


# --------------------------------------------------------------------------
# Host-side routing / layout
# --------------------------------------------------------------------------

def _np16():
    if os.environ.get("KERNEL_DT", "float16") == "bfloat16":
        import ml_dtypes
        return ml_dtypes.bfloat16
    return np.float16


def _fm(w, dtype):
    """[K, ...] -> [128, K//128, ...] with contraction index f = o*128 + p."""
    ko = w.shape[0] // 128
    perm = (1, 0) + tuple(range(2, w.ndim + 1))
    return np.ascontiguousarray(
        w.reshape(ko, 128, *w.shape[1:]).transpose(*perm), dtype=dtype)


def _prep_weights(i):
    f32 = np.float32
    f16 = _np16()

    def a(x):
        return np.asarray(x, dtype=f32)

    wb1 = a(i["Wb1"]).transpose(1, 0, 2)   # [640, 6, 256]
    wb2 = a(i["Wb2"]).transpose(1, 0, 2)   # [256, 6, 256]
    wb3 = a(i["Wb3"]).transpose(1, 0, 2)   # [256, 6, 3]
    bb1 = a(i["bb1"]).T                    # [256, 6]
    bb2 = a(i["bb2"]).T
    return {
        "wsi1": np.ascontiguousarray(a(i["Wsi1"]), dtype=f16),   # [1, 256]
        "bsi1": _fm(a(i["bsi1"]), f32),                          # [128, 2]
        "wsi2": _fm(a(i["Wsi2"]), f16),                          # [128, 2, 128]
        "bsi2": a(i["bsi2"]).reshape(128, 1).copy(),             # [128, 1]
        "wso1": _fm(a(i["Wso1"]), f16),                          # [128, 5, 256]
        "bso1": _fm(a(i["bso1"]), f32),                          # [128, 2]
        "wso2": _fm(a(i["Wso2"]), f16),                          # [128, 2, 256]
        "bso2": _fm(a(i["bso2"]), f32),                          # [128, 2]
        "wso3": _fm(a(i["Wso3"]), f16),                          # [128, 2, 1]
        "bso3": a(i["bso3"]).reshape(1, 1).copy(),               # [1, 1]
        "wb1": _fm(wb1, f16),                                    # [128, 5, 6, 256]
        "bb1": _fm(bb1, f32),                                    # [128, 2, 6]
        "wb2": _fm(wb2, f16),                                    # [128, 2, 6, 256]
        "bb2": _fm(bb2, f32),                                    # [128, 2, 6]
        "wb3": _fm(wb3, f16),                                    # [128, 2, 6, 3]
        "bb3": np.ascontiguousarray(a(i["bb3"]).T),              # [3, 6]
    }


def _route(cmd):
    """Assign rows to (core, slot-position); slot k of every core holds only
    branch-k rows. Returns idx [NCORES, BPC], valid [NCORES, BPC], spill."""
    idx = np.zeros((NCORES, BPC), dtype=np.int64)
    valid = np.zeros((NCORES, BPC), dtype=bool)
    spill = []
    for k in range(NBRANCH):
        rows = np.flatnonzero(cmd == k)
        for c, part in enumerate(np.array_split(rows, NCORES)):
            if len(part) > CAP:
                spill.append(part[CAP:])
                part = part[:CAP]
            idx[c, k * CAP:k * CAP + len(part)] = part
            valid[c, k * CAP:k * CAP + len(part)] = True
    spill = np.concatenate(spill) if spill else np.zeros(0, dtype=np.int64)
    return idx, valid, spill


def _np_reference(i, rows):
    """Exact reference math in numpy for a subset of rows (spill fallback)."""
    f32 = np.float32
    E = np.asarray(i["embedding"], f32)[rows]
    S = np.asarray(i["speed"], f32)[rows]
    cmd = np.asarray(i["command"])[rows].astype(np.int64) - 1
    sp = np.maximum(S @ np.asarray(i["Wsi1"], f32) + np.asarray(i["bsi1"], f32), 0)
    sp = sp @ np.asarray(i["Wsi2"], f32) + np.asarray(i["bsi2"], f32)
    emb = np.concatenate([E, sp], axis=1)
    hs = np.maximum(emb @ np.asarray(i["Wso1"], f32) + np.asarray(i["bso1"], f32), 0)
    hs = np.maximum(hs @ np.asarray(i["Wso2"], f32) + np.asarray(i["bso2"], f32), 0)
    spd = hs @ np.asarray(i["Wso3"], f32) + np.asarray(i["bso3"], f32)
    ctrl = np.zeros((len(rows), 3), f32)
    for k in range(NBRANCH):
        m = cmd == k
        if not m.any():
            continue
        h = np.maximum(emb[m] @ np.asarray(i["Wb1"], f32)[k]
                       + np.asarray(i["bb1"], f32)[k], 0)
        h = np.maximum(h @ np.asarray(i["Wb2"], f32)[k]
                       + np.asarray(i["bb2"], f32)[k], 0)
        z = h @ np.asarray(i["Wb3"], f32)[k] + np.asarray(i["bb3"], f32)[k]
        ctrl[m] = 1.0 / (1.0 + np.exp(-z))
    return ctrl, spd.astype(f32)


# --------------------------------------------------------------------------
# Entry point
# --------------------------------------------------------------------------

LAST_RESULTS = None  # BassKernelResults of the most recent device run


def kernel(embedding, speed, command, **weights):
    global LAST_RESULTS
    inputs = dict(weights)
    inputs.update(embedding=embedding, speed=speed, command=command)

    embedding = np.asarray(embedding, dtype=np.float32)
    speed = np.asarray(speed, dtype=np.float32)
    command_np = np.asarray(command)

    if embedding.shape != (B, D_EMB):
        # Unexpected problem size: fall back to exact host computation.
        ctrl, spd = _np_reference(inputs, np.arange(embedding.shape[0]))
        return ctrl, spd

    cmd = command_np.astype(np.int64) - 1
    idx, valid, spill = _route(cmd)

    w = _prep_weights(inputs)

    in_maps = []
    for c in range(NCORES):
        rows = idx[c]
        xt = np.empty((D_EMB + 1, BPC), dtype=_np16())
        xt[:D_EMB] = embedding[rows].T
        xt[D_EMB] = speed[rows, 0]
        m = {"xt": xt}
        m.update(w)
        in_maps.append(m)

    from concourse.bass_utils import run_bass_kernel_spmd

    nc = _build_nc()
    res = run_bass_kernel_spmd(
        nc, in_maps, core_ids=list(range(NCORES)),
        trace=bool(int(os.environ.get("KERNEL_TRACE", "0"))),
    )
    LAST_RESULTS = res

    control = np.zeros((B, 3), dtype=np.float32)
    speed_pred = np.zeros((B, 1), dtype=np.float32)
    for c in range(NCORES):
        o4 = np.asarray(res.results[c]["out4"])
        v = valid[c]
        rows = idx[c][v]
        control[rows] = o4[0:3, v].T
        speed_pred[rows, 0] = o4[3, v]

    if len(spill):
        ctrl_sp, spd_sp = _np_reference(inputs, spill)
        control[spill] = ctrl_sp
        speed_pred[spill] = spd_sp

    return control, speed_pred


# revision 15
# speedup vs baseline: 1.1221x; 1.1221x over previous
"""Trainium2 Bass kernel for nn_CILRSModel (moe_routing).

Strategy:
  - Host-side MoE routing: rows are bucketed by `command` (6 branches) and
    distributed evenly over 8 cores. Each core gets a fixed [6 x CAP] row
    layout so the SPMD kernel statically knows which branch weights apply
    to which batch tile (no on-device routing control flow).
  - Host-side transpose: the embedding is shipped feature-major (partition =
    feature) so every matmul operand already has the contraction dim on SBUF
    partitions - zero on-device transposes. The speed scalar is shipped
    broadcast across all 128 partitions so the speed-MLP's first (K=1) layer
    runs on the Scalar engine (per-partition scale+bias+relu), not the PE.
  - On device everything is feature-major fp16 (PSUM accumulates fp32):
    x [640, N] tiles flow through the speed-MLP, speed head and the single
    routed branch head on the PE; PSUM is evicted with fused bias+ReLU
    (tensor_scalar / activation) split across Vector and Scalar engines.
  - The two tiny-M output matmuls (control M=3, speed M=1) are packed into
    distinct PE column groups via tile_position so they run concurrently.
  - Outputs return feature-major as out4 = [4, rows] and are scattered back
    to the original row order on host.
"""

import os
import sys

import numpy as np

_TRN_REPO = "/opt/trn_rl_repo"
if _TRN_REPO not in sys.path:
    sys.path.insert(0, _TRN_REPO)

# Problem constants (hardcoded per harness contract)
B = 65536
D_EMB = 512
D_LAT = 128
H = 256
NBRANCH = 6
D_IN = D_EMB + D_LAT  # 640
NCORES = 8
CAP = 1408            # per-core per-branch row capacity (actual max ~1389)
BPC = NBRANCH * CAP   # 8448 rows per core
# batch tiles per branch slot: 512 + 512 + 384 columns
SLOT_TILES = [(0, 512), (512, 512), (1024, 384)]
PACK_TAIL = True      # pack ctrl (M=3) + spd (M=1) matmuls into col groups

_cache = {}


# --------------------------------------------------------------------------
# Device kernel
# --------------------------------------------------------------------------

def _build_nc():
    if "nc" in _cache:
        return _cache["nc"]

    import concourse.mybir as mybir
    import concourse.tile as tile
    from concourse import bacc

    f32 = mybir.dt.float32
    f16 = getattr(mybir.dt, os.environ.get("KERNEL_DT", "float16"))
    AF = mybir.ActivationFunctionType
    ALU = mybir.AluOpType

    nc = bacc.Bacc("TRN2", target_bir_lowering=False, debug=False,
                   num_devices=NCORES)

    xt = nc.dram_tensor("xt", [128, 5, BPC], f16, kind="ExternalInput")[:]
    wa = nc.dram_tensor("wa", [128, 2050], f16, kind="ExternalInput")[:]
    wb = nc.dram_tensor("wb", [128, 10788], f16, kind="ExternalInput")[:]
    wc = nc.dram_tensor("wc", [128, 40], f32, kind="ExternalInput")[:]
    out4 = nc.dram_tensor("out4", [4, BPC], f32, kind="ExternalOutput")[:]

    with tile.TileContext(nc) as tc:
        with (
            tc.tile_pool(name="wpool", bufs=1) as wpool,
            tc.tile_pool(name="xpool", bufs=4) as xpool,
            tc.tile_pool(name="hpool", bufs=2) as hpool,
            tc.tile_pool(name="opool", bufs=1) as opool,
            tc.tile_pool(name="pmm", bufs=3, space="PSUM") as pmm,
            tc.tile_pool(name="pm1", bufs=2, space="PSUM") as pm1,
        ):
            wc_s = wpool.tile([128, 40], f32, tag="wc")
            nc.sync.dma_start(out=wc_s[:], in_=wc)
            wa_s = wpool.tile([128, 2050], f16, tag="wa")
            nc.sync.dma_start(out=wa_s[:], in_=wa)
            # branch weights come in via SWDGE so they don't delay the
            # x-tile loads queued on the Sync HWDGE path
            wb_s = wpool.tile([128, 10788], f16, tag="wb")
            nc.gpsimd.dma_start(out=wb_s[:], in_=wb)

            # f16 blob A slices
            def wsi2_sl(o):
                return wa_s[:, o * 128:(o + 1) * 128]

            def wso1_sl(o, j):
                c = 256 + o * 256 + j * 128
                return wa_s[:, c:c + 128]

            def wso2_sl(o, j):
                c = 1536 + o * 256 + j * 128
                return wa_s[:, c:c + 128]

            def wso3_sl(o):
                return wa_s[:, 2048 + o:2049 + o]

            # f16 blob B slices
            def wb1_sl(o, k, j):
                c = o * 1536 + k * 256 + j * 128
                return wb_s[:, c:c + 128]

            def wb2_sl(o, k, j):
                c = 7680 + o * 1536 + k * 256 + j * 128
                return wb_s[:, c:c + 128]

            def wb3_sl(o, k):
                c = 10752 + o * 18 + k * 3
                return wb_s[:, c:c + 3]

            # f32 blob slices (per-partition bias/scale columns)
            def bsi1_c(j):
                return wc_s[:, j:j + 1]

            bsi2_c = wc_s[:, 2:3]

            def bso1_c(j):
                return wc_s[:, 3 + j:4 + j]

            def bso2_c(j):
                return wc_s[:, 5 + j:6 + j]

            def wsi1_c(j):
                return wc_s[:, 7 + j:8 + j]

            def bb1_c(j, k):
                c = 9 + j * 6 + k
                return wc_s[:, c:c + 1]

            def bb2_c(j, k):
                c = 21 + j * 6 + k
                return wc_s[:, c:c + 1]

            bso3_c = wc_s[0:1, 33:34]
            bb3_c = wc_s[32:35, 34:40]   # bb3 lives at partitions 32:35

            ctl_s = opool.tile([35, BPC], f32, tag="octl")  # rows 32:35 used
            spd_s = opool.tile([1, BPC], f32, tag="ospd")

            def evict_relu(dst, src, bias_ap, on_act):
                if on_act:
                    nc.scalar.activation(dst, src, AF.Relu, bias=bias_ap)
                else:
                    nc.vector.tensor_scalar(dst, src, bias_ap, 0.0, ALU.add, ALU.max)

            for k in range(NBRANCH):
                for off, nt in SLOT_TILES:
                    c0 = k * CAP + off
                    cols = slice(c0, c0 + nt)

                    x_s = xpool.tile([128, 5, nt], f16, tag=f"x{nt}")
                    nc.sync.dma_start(out=x_s[:], in_=xt[:, :, cols])

                    # ---- speed-input MLP layer 1 on ScalarE:
                    # relu(speed * wsi1[f] + bsi1[f]) with per-partition
                    # scale/bias, input = host-broadcast speed row
                    hsp = hpool.tile([128, 2, nt], f16, tag=f"hsp{nt}")
                    for j in range(2):
                        nc.scalar.activation(hsp[:, j, :], x_s[:, 4, :], AF.Relu,
                                             bias=bsi1_c(j), scale=wsi1_c(j))

                    # ---- speed head layer 1, emb subtiles only (o=0..3)
                    p1 = pmm.tile([128, 2, 512], f32, tag="pmm")
                    for j in range(2):
                        for o in range(4):
                            nc.tensor.matmul(p1[:, j, :nt], wso1_sl(o, j),
                                             x_s[:, o, :],
                                             start=(o == 0), stop=False)

                    # ---- speed-input MLP layer 2 -> latent into x_s[:,4,:]
                    p_sp = pm1.tile([128, 512], f32, tag="pm1")
                    nc.tensor.matmul(p_sp[:, :nt], wsi2_sl(0), hsp[:, 0, :],
                                     start=True, stop=False)
                    nc.tensor.matmul(p_sp[:, :nt], wsi2_sl(1), hsp[:, 1, :],
                                     start=False, stop=True)

                    # ---- branch layer 1, emb subtiles only
                    q1 = pmm.tile([128, 2, 512], f32, tag="pmm")
                    for j in range(2):
                        for o in range(4):
                            nc.tensor.matmul(q1[:, j, :nt], wb1_sl(o, k, j),
                                             x_s[:, o, :],
                                             start=(o == 0), stop=False)

                    # speed latent eviction (bias add, no relu)
                    nc.vector.tensor_scalar(x_s[:, 4, :], p_sp[:, :nt], bsi2_c,
                                            None, ALU.add)

                    # ---- finish layer-1 accumulations with latent subtile
                    for j in range(2):
                        nc.tensor.matmul(p1[:, j, :nt], wso1_sl(4, j),
                                         x_s[:, 4, :], start=False, stop=True)
                    for j in range(2):
                        nc.tensor.matmul(q1[:, j, :nt], wb1_sl(4, k, j),
                                         x_s[:, 4, :], start=False, stop=True)

                    h1 = hpool.tile([128, 2, nt], f16, tag=f"h1{nt}")
                    evict_relu(h1[:, 0, :], p1[:, 0, :nt], bso1_c(0), True)
                    evict_relu(h1[:, 1, :], p1[:, 1, :nt], bso1_c(1), False)

                    # ---- speed head layer 2
                    p2 = pmm.tile([128, 2, 512], f32, tag="pmm")
                    for j in range(2):
                        for o in range(2):
                            nc.tensor.matmul(p2[:, j, :nt], wso2_sl(o, j),
                                             h1[:, o, :],
                                             start=(o == 0), stop=(o == 1))

                    g1 = hpool.tile([128, 2, nt], f16, tag=f"g1{nt}")
                    evict_relu(g1[:, 0, :], q1[:, 0, :nt], bb1_c(0, k), True)
                    evict_relu(g1[:, 1, :], q1[:, 1, :nt], bb1_c(1, k), False)

                    # ---- branch layer 2
                    q2 = pmm.tile([128, 2, 512], f32, tag="pmm")
                    for j in range(2):
                        for o in range(2):
                            nc.tensor.matmul(q2[:, j, :nt], wb2_sl(o, k, j),
                                             g1[:, o, :],
                                             start=(o == 0), stop=(o == 1))

                    h2 = hpool.tile([128, 2, nt], f16, tag=f"h2{nt}")
                    evict_relu(h2[:, 0, :], p2[:, 0, :nt], bso2_c(0), True)
                    evict_relu(h2[:, 1, :], p2[:, 1, :nt], bso2_c(1), False)

                    g2 = hpool.tile([128, 2, nt], f16, tag=f"g2{nt}")
                    evict_relu(g2[:, 0, :], q2[:, 0, :nt], bb2_c(0, k), True)
                    evict_relu(g2[:, 1, :], q2[:, 1, :nt], bb2_c(1, k), False)

                    # ---- outputs: ctrl (M=3) and spd (M=1), col-group packed
                    if PACK_TAIL:
                        # spd (M=1) in col group 0, ctrl (M=3) in col group 1;
                        # all downstream APs stay partition-base aligned.
                        p_cs = pm1.tile([64, 512], f32, tag="pm1")
                        nc.tensor.matmul(p_cs[0:1, :nt], wso3_sl(0), h2[:, 0, :],
                                         start=True, stop=False,
                                         tile_position=(0, 0))
                        nc.tensor.matmul(p_cs[0:1, :nt], wso3_sl(1), h2[:, 1, :],
                                         start=False, stop=True,
                                         tile_position=(0, 0))
                        nc.tensor.matmul(p_cs[32:35, :nt], wb3_sl(0, k), g2[:, 0, :],
                                         start=True, stop=False,
                                         tile_position=(0, 32))
                        nc.tensor.matmul(p_cs[32:35, :nt], wb3_sl(1, k), g2[:, 1, :],
                                         start=False, stop=True,
                                         tile_position=(0, 32))
                        nc.vector.tensor_scalar(spd_s[:, cols], p_cs[0:1, :nt],
                                                bso3_c, None, ALU.add)
                        nc.scalar.activation(ctl_s[32:35, cols], p_cs[32:35, :nt],
                                             AF.Sigmoid, bias=bb3_c[:, k:k + 1])
                    else:
                        p_o = pm1.tile([1, 512], f32, tag="pm1")
                        nc.tensor.matmul(p_o[:, :nt], wso3_sl(0), h2[:, 0, :],
                                         start=True, stop=False)
                        nc.tensor.matmul(p_o[:, :nt], wso3_sl(1), h2[:, 1, :],
                                         start=False, stop=True)
                        nc.vector.tensor_scalar(spd_s[:, cols], p_o[:, :nt],
                                                bso3_c, None, ALU.add)
                        p_c = pm1.tile([3, 512], f32, tag="pm1")
                        nc.tensor.matmul(p_c[:, :nt], wb3_sl(0, k), g2[:, 0, :],
                                         start=True, stop=False)
                        nc.tensor.matmul(p_c[:, :nt], wb3_sl(1, k), g2[:, 1, :],
                                         start=False, stop=True)
                        nc.scalar.activation(ctl_s[32:35, cols], p_c[:, :nt],
                                             AF.Sigmoid, bias=bb3_c[:, k:k + 1])

            nc.sync.dma_start(out=out4[0:3, :], in_=ctl_s[32:35, :])
            nc.sync.dma_start(out=out4[3:4, :], in_=spd_s[:])

    nc.compile()
    _cache["nc"] = nc
    return nc


# --------------------------------------------------------------------------
# Host-side routing / layout
# --------------------------------------------------------------------------

def _np16():
    if os.environ.get("KERNEL_DT", "float16") == "bfloat16":
        import ml_dtypes
        return ml_dtypes.bfloat16
    return np.float16


def _fm(w, dtype):
    """[K, ...] -> [128, K//128, ...] with contraction index f = o*128 + p."""
    ko = w.shape[0] // 128
    perm = (1, 0) + tuple(range(2, w.ndim + 1))
    return np.ascontiguousarray(
        w.reshape(ko, 128, *w.shape[1:]).transpose(*perm), dtype=dtype)


def _prep_weights(i):
    f32 = np.float32
    f16 = _np16()

    def a(x):
        return np.asarray(x, dtype=f32)

    # f16 blob A: [wsi2 | wso1 | wso2 | wso3] along free dim
    wsi2 = _fm(a(i["Wsi2"]), f16).reshape(128, -1)            # 256
    wso1 = _fm(a(i["Wso1"]), f16).reshape(128, -1)            # 1280
    wso2 = _fm(a(i["Wso2"]), f16).reshape(128, -1)            # 512
    wso3 = _fm(a(i["Wso3"]), f16).reshape(128, -1)            # 2
    wa = np.concatenate([wsi2, wso1, wso2, wso3], axis=1)
    assert wa.shape == (128, 2050)

    # f16 blob B: [wb1 | wb2 | wb3]
    wb1 = _fm(a(i["Wb1"]).transpose(1, 0, 2), f16).reshape(128, -1)   # 7680
    wb2 = _fm(a(i["Wb2"]).transpose(1, 0, 2), f16).reshape(128, -1)   # 3072
    wb3 = _fm(a(i["Wb3"]).transpose(1, 0, 2), f16).reshape(128, -1)   # 36
    wb = np.concatenate([wb1, wb2, wb3], axis=1)
    assert wb.shape == (128, 10788), wb.shape

    # f32 blob: per-partition bias/scale columns
    wc = np.zeros((128, 40), dtype=f32)
    wc[:, 0:2] = _fm(a(i["bsi1"]), f32)                 # bsi1
    wc[:, 2] = a(i["bsi2"])                             # bsi2
    wc[:, 3:5] = _fm(a(i["bso1"]), f32)                 # bso1
    wc[:, 5:7] = _fm(a(i["bso2"]), f32)                 # bso2
    wc[:, 7:9] = _fm(a(i["Wsi1"]).reshape(256), f32)    # wsi1 as scale
    wc[:, 9:21] = _fm(a(i["bb1"]).T, f32).reshape(128, 12)   # bb1 [p, j*6+k]
    wc[:, 21:33] = _fm(a(i["bb2"]).T, f32).reshape(128, 12)  # bb2
    wc[0, 33] = a(i["bso3"])[0]                         # bso3 (partition 0)
    wc[32:35, 34:40] = a(i["bb3"]).T                    # bb3 (partitions 32:35)
    return {"wa": wa, "wb": wb, "wc": wc}


def _route(cmd):
    """Assign rows to (core, slot-position); slot k of every core holds only
    branch-k rows. Returns idx [NCORES, BPC], valid [NCORES, BPC], spill."""
    idx = np.zeros((NCORES, BPC), dtype=np.int64)
    valid = np.zeros((NCORES, BPC), dtype=bool)
    spill = []
    for k in range(NBRANCH):
        rows = np.flatnonzero(cmd == k)
        for c, part in enumerate(np.array_split(rows, NCORES)):
            if len(part) > CAP:
                spill.append(part[CAP:])
                part = part[:CAP]
            idx[c, k * CAP:k * CAP + len(part)] = part
            valid[c, k * CAP:k * CAP + len(part)] = True
    spill = np.concatenate(spill) if spill else np.zeros(0, dtype=np.int64)
    return idx, valid, spill


def _np_reference(i, rows):
    """Exact reference math in numpy for a subset of rows (spill fallback)."""
    f32 = np.float32
    E = np.asarray(i["embedding"], f32)[rows]
    S = np.asarray(i["speed"], f32)[rows]
    cmd = np.asarray(i["command"])[rows].astype(np.int64) - 1
    sp = np.maximum(S @ np.asarray(i["Wsi1"], f32) + np.asarray(i["bsi1"], f32), 0)
    sp = sp @ np.asarray(i["Wsi2"], f32) + np.asarray(i["bsi2"], f32)
    emb = np.concatenate([E, sp], axis=1)
    hs = np.maximum(emb @ np.asarray(i["Wso1"], f32) + np.asarray(i["bso1"], f32), 0)
    hs = np.maximum(hs @ np.asarray(i["Wso2"], f32) + np.asarray(i["bso2"], f32), 0)
    spd = hs @ np.asarray(i["Wso3"], f32) + np.asarray(i["bso3"], f32)
    ctrl = np.zeros((len(rows), 3), f32)
    for k in range(NBRANCH):
        m = cmd == k
        if not m.any():
            continue
        h = np.maximum(emb[m] @ np.asarray(i["Wb1"], f32)[k]
                       + np.asarray(i["bb1"], f32)[k], 0)
        h = np.maximum(h @ np.asarray(i["Wb2"], f32)[k]
                       + np.asarray(i["bb2"], f32)[k], 0)
        z = h @ np.asarray(i["Wb3"], f32)[k] + np.asarray(i["bb3"], f32)[k]
        ctrl[m] = 1.0 / (1.0 + np.exp(-z))
    return ctrl, spd.astype(f32)


# --------------------------------------------------------------------------
# Entry point
# --------------------------------------------------------------------------

LAST_RESULTS = None  # BassKernelResults of the most recent device run


def kernel(embedding, speed, command, **weights):
    global LAST_RESULTS
    inputs = dict(weights)
    inputs.update(embedding=embedding, speed=speed, command=command)

    embedding = np.asarray(embedding, dtype=np.float32)
    speed = np.asarray(speed, dtype=np.float32)
    command_np = np.asarray(command)

    if embedding.shape != (B, D_EMB):
        # Unexpected problem size: fall back to exact host computation.
        ctrl, spd = _np_reference(inputs, np.arange(embedding.shape[0]))
        return ctrl, spd

    cmd = command_np.astype(np.int64) - 1
    idx, valid, spill = _route(cmd)

    w = _prep_weights(inputs)
    f16 = _np16()

    in_maps = []
    for c in range(NCORES):
        rows = idx[c]
        xt = np.empty((128, 5, BPC), dtype=f16)
        emb_t = embedding[rows].T.astype(f16)               # [512, BPC]
        xt[:, 0:4, :] = emb_t.reshape(4, 128, BPC).transpose(1, 0, 2)
        xt[:, 4, :] = speed[rows, 0].astype(f16)[None, :]
        m = {"xt": xt}
        m.update(w)
        in_maps.append(m)

    from concourse.bass_utils import run_bass_kernel_spmd

    nc = _build_nc()
    res = run_bass_kernel_spmd(
        nc, in_maps, core_ids=list(range(NCORES)),
        trace=bool(int(os.environ.get("KERNEL_TRACE", "0"))),
    )
    LAST_RESULTS = res

    control = np.zeros((B, 3), dtype=np.float32)
    speed_pred = np.zeros((B, 1), dtype=np.float32)
    for c in range(NCORES):
        o4 = np.asarray(res.results[c]["out4"])
        v = valid[c]
        rows = idx[c][v]
        control[rows] = o4[0:3, v].T
        speed_pred[rows, 0] = o4[3, v]

    if len(spill):
        ctrl_sp, spd_sp = _np_reference(inputs, spill)
        control[spill] = ctrl_sp
        speed_pred[spill] = spd_sp

    return control, speed_pred


# revision 16
# speedup vs baseline: 1.1959x; 1.0657x over previous
"""Trainium2 Bass kernel for nn_CILRSModel (moe_routing).

Strategy:
  - Host-side MoE routing: rows are bucketed by `command` (6 branches) and
    distributed evenly over 8 cores. Each core gets a fixed [6 x CAP] row
    layout so the SPMD kernel statically knows which branch weights apply
    to which batch tile (no on-device routing control flow).
  - Host-side transpose: the embedding is shipped feature-major (partition =
    feature), per batch-tile contiguous, so every matmul operand already has
    the contraction dim on SBUF partitions and every x load is one fast DMA.
    The speed scalar is shipped broadcast across all 128 partitions so the
    speed-MLP's first (K=1) layer runs on the Scalar engine.
  - On device everything is feature-major fp16 (PSUM accumulates fp32).
    Two-stage software pipeline: tile t+1's first-layer matmuls are emitted
    before tile t's second/third layers so eviction latency hides behind
    independent PE work.
  - The two tiny-M output matmuls (control M=3, speed M=1) are packed into
    distinct PE column groups via tile_position so they run concurrently.
  - Outputs return feature-major as out4 = [4, rows] and are scattered back
    to the original row order on host.
"""

import os
import sys

import numpy as np

_TRN_REPO = "/opt/trn_rl_repo"
if _TRN_REPO not in sys.path:
    sys.path.insert(0, _TRN_REPO)

# Problem constants (hardcoded per harness contract)
B = 65536
D_EMB = 512
D_LAT = 128
H = 256
NBRANCH = 6
D_IN = D_EMB + D_LAT  # 640
NCORES = 8
CAP = 1408            # per-core per-branch row capacity (actual max ~1389)
BPC = NBRANCH * CAP   # 8448 rows per core
SLOT_TILES = [(0, 512), (512, 512), (1024, 384)]
WB_K = 1798           # per-branch weight-blob width: 5*256 + 2*256 + 2*3

# tile table: (branch k, col offset, width, xt element offset)
TILES = []
_eoff = 0
for _k in range(NBRANCH):
    for _off, _nt in SLOT_TILES:
        TILES.append((_k, _off, _nt, _eoff))
        _eoff += 128 * 5 * _nt
XT_ELEMS = _eoff

_cache = {}


# --------------------------------------------------------------------------
# Device kernel
# --------------------------------------------------------------------------

def _build_nc():
    if "nc" in _cache:
        return _cache["nc"]

    import concourse.mybir as mybir
    import concourse.tile as tile
    from concourse import bacc

    f32 = mybir.dt.float32
    f16 = getattr(mybir.dt, os.environ.get("KERNEL_DT", "float16"))
    AF = mybir.ActivationFunctionType
    ALU = mybir.AluOpType

    nc = bacc.Bacc("TRN2", target_bir_lowering=False, debug=False,
                   num_devices=NCORES)

    xt = nc.dram_tensor("xt", [XT_ELEMS], f16, kind="ExternalInput")[:]
    wa = nc.dram_tensor("wa", [128, 2050], f16, kind="ExternalInput")[:]
    wb = nc.dram_tensor("wb", [128, NBRANCH * WB_K], f16, kind="ExternalInput")[:]
    wc = nc.dram_tensor("wc", [128, 40], f32, kind="ExternalInput")[:]
    out4 = nc.dram_tensor("out4", [4, BPC], f32, kind="ExternalOutput")[:]

    with tile.TileContext(nc) as tc:
        with (
            tc.tile_pool(name="wpool", bufs=1) as wpool,
            tc.tile_pool(name="xpool", bufs=4) as xpool,
            tc.tile_pool(name="hpool", bufs=2) as hpool,
            tc.tile_pool(name="opool", bufs=1) as opool,
            tc.tile_pool(name="pmm", bufs=3, space="PSUM") as pmm,
            tc.tile_pool(name="pm1", bufs=2, space="PSUM") as pm1,
        ):
            wc_s = wpool.tile([128, 40], f32, tag="wc")
            nc.sync.dma_start(out=wc_s[:], in_=wc)
            wa_s = wpool.tile([128, 2050], f16, tag="wa")
            nc.sync.dma_start(out=wa_s[:], in_=wa)
            wb_s = wpool.tile([128, NBRANCH * WB_K], f16, tag="wb")
            # branch 0 weights up front; later branches prefetched in-loop
            nc.sync.dma_start(out=wb_s[:, 0:WB_K], in_=wb[:, 0:WB_K])

            # f16 blob A slices
            def wsi2_sl(o):
                return wa_s[:, o * 128:(o + 1) * 128]

            def wso1_sl(o, j):
                c = 256 + o * 256 + j * 128
                return wa_s[:, c:c + 128]

            def wso2_sl(o, j):
                c = 1536 + o * 256 + j * 128
                return wa_s[:, c:c + 128]

            def wso3_sl(o):
                return wa_s[:, 2048 + o:2049 + o]

            # f16 blob B slices (per-branch-contiguous)
            def wb1_sl(o, k, j):
                c = k * WB_K + o * 256 + j * 128
                return wb_s[:, c:c + 128]

            def wb2_sl(o, k, j):
                c = k * WB_K + 1280 + o * 256 + j * 128
                return wb_s[:, c:c + 128]

            def wb3_sl(o, k):
                c = k * WB_K + 1792 + o * 3
                return wb_s[:, c:c + 3]

            # f32 blob slices (per-partition bias/scale columns)
            def bsi1_c(j):
                return wc_s[:, j:j + 1]

            bsi2_c = wc_s[:, 2:3]

            def bso1_c(j):
                return wc_s[:, 3 + j:4 + j]

            def bso2_c(j):
                return wc_s[:, 5 + j:6 + j]

            def wsi1_c(j):
                return wc_s[:, 7 + j:8 + j]

            def bb1_c(j, k):
                c = 9 + j * 6 + k
                return wc_s[:, c:c + 1]

            def bb2_c(j, k):
                c = 21 + j * 6 + k
                return wc_s[:, c:c + 1]

            bso3_c = wc_s[0:1, 33:34]
            bb3_c = wc_s[32:35, 34:40]   # bb3 lives at partitions 32:35

            ctl_s = opool.tile([35, BPC], f32, tag="octl")  # rows 32:35 used
            spd_s = opool.tile([1, BPC], f32, tag="ospd")

            def evict_relu(dst, src, bias_ap, on_act):
                if on_act:
                    nc.scalar.activation(dst, src, AF.Relu, bias=bias_ap)
                else:
                    nc.vector.tensor_scalar(dst, src, bias_ap, 0.0, ALU.add, ALU.max)

            def stage_a(t):
                """x load, speed MLP, layer-1 of both heads, h1/g1 evictions."""
                k, off, nt, eoff = TILES[t]

                x_s = xpool.tile([128, 5, nt], f16, tag=f"x{nt}")
                src = xt[eoff:eoff + 128 * 5 * nt].rearrange(
                    "(p o b) -> p o b", p=128, o=5)
                nc.sync.dma_start(out=x_s[:], in_=src)

                # prefetch the next branch's weights at each slot start
                if off == 0 and k + 1 < NBRANCH:
                    c = (k + 1) * WB_K
                    nc.sync.dma_start(out=wb_s[:, c:c + WB_K],
                                      in_=wb[:, c:c + WB_K])

                # speed-MLP layer 1 on ScalarE (per-partition scale+bias+relu)
                hsp = hpool.tile([128, 2, nt], f16, tag=f"hsp{nt}")
                for j in range(2):
                    nc.scalar.activation(hsp[:, j, :], x_s[:, 4, :], AF.Relu,
                                         bias=bsi1_c(j), scale=wsi1_c(j))

                # layer-1 emb subtiles (o=0..3) of both heads
                p1 = pmm.tile([128, 2, 512], f32, tag="pmm")
                for j in range(2):
                    for o in range(4):
                        nc.tensor.matmul(p1[:, j, :nt], wso1_sl(o, j),
                                         x_s[:, o, :],
                                         start=(o == 0), stop=False)
                q1 = pmm.tile([128, 2, 512], f32, tag="pmm")
                for j in range(2):
                    for o in range(4):
                        nc.tensor.matmul(q1[:, j, :nt], wb1_sl(o, k, j),
                                         x_s[:, o, :],
                                         start=(o == 0), stop=False)

                # speed-MLP layer 2 -> latent into x_s[:,4,:]
                p_sp = pm1.tile([128, 512], f32, tag="pm1")
                nc.tensor.matmul(p_sp[:, :nt], wsi2_sl(0), hsp[:, 0, :],
                                 start=True, stop=False)
                nc.tensor.matmul(p_sp[:, :nt], wsi2_sl(1), hsp[:, 1, :],
                                 start=False, stop=True)
                nc.vector.tensor_scalar(x_s[:, 4, :], p_sp[:, :nt], bsi2_c,
                                        None, ALU.add)

                # finish layer-1 accumulations with the latent subtile
                for j in range(2):
                    nc.tensor.matmul(p1[:, j, :nt], wso1_sl(4, j),
                                     x_s[:, 4, :], start=False, stop=True)
                for j in range(2):
                    nc.tensor.matmul(q1[:, j, :nt], wb1_sl(4, k, j),
                                     x_s[:, 4, :], start=False, stop=True)

                h1 = hpool.tile([128, 2, nt], f16, tag=f"h1{nt}")
                evict_relu(h1[:, 0, :], p1[:, 0, :nt], bso1_c(0), True)
                evict_relu(h1[:, 1, :], p1[:, 1, :nt], bso1_c(1), False)
                g1 = hpool.tile([128, 2, nt], f16, tag=f"g1{nt}")
                evict_relu(g1[:, 0, :], q1[:, 0, :nt], bb1_c(0, k), True)
                evict_relu(g1[:, 1, :], q1[:, 1, :nt], bb1_c(1, k), False)
                return (k, off, nt, h1, g1)

            def stage_b(st):
                """Layer-2 of both heads, output layer, final evictions."""
                k, off, nt, h1, g1 = st
                c0 = k * CAP + off
                cols = slice(c0, c0 + nt)

                p2 = pmm.tile([128, 2, 512], f32, tag="pmm")
                for j in range(2):
                    for o in range(2):
                        nc.tensor.matmul(p2[:, j, :nt], wso2_sl(o, j),
                                         h1[:, o, :],
                                         start=(o == 0), stop=(o == 1))
                q2 = pmm.tile([128, 2, 512], f32, tag="pmm")
                for j in range(2):
                    for o in range(2):
                        nc.tensor.matmul(q2[:, j, :nt], wb2_sl(o, k, j),
                                         g1[:, o, :],
                                         start=(o == 0), stop=(o == 1))

                h2 = hpool.tile([128, 2, nt], f16, tag=f"h2{nt}")
                evict_relu(h2[:, 0, :], p2[:, 0, :nt], bso2_c(0), True)
                evict_relu(h2[:, 1, :], p2[:, 1, :nt], bso2_c(1), False)
                g2 = hpool.tile([128, 2, nt], f16, tag=f"g2{nt}")
                evict_relu(g2[:, 0, :], q2[:, 0, :nt], bb2_c(0, k), False)
                evict_relu(g2[:, 1, :], q2[:, 1, :nt], bb2_c(1, k), False)

                # outputs: spd (M=1) in col group 0, ctrl (M=3) in col group 1
                p_cs = pm1.tile([64, 512], f32, tag="pm1")
                nc.tensor.matmul(p_cs[0:1, :nt], wso3_sl(0), h2[:, 0, :],
                                 start=True, stop=False, tile_position=(0, 0))
                nc.tensor.matmul(p_cs[0:1, :nt], wso3_sl(1), h2[:, 1, :],
                                 start=False, stop=True, tile_position=(0, 0))
                nc.tensor.matmul(p_cs[32:35, :nt], wb3_sl(0, k), g2[:, 0, :],
                                 start=True, stop=False, tile_position=(0, 32))
                nc.tensor.matmul(p_cs[32:35, :nt], wb3_sl(1, k), g2[:, 1, :],
                                 start=False, stop=True, tile_position=(0, 32))
                nc.vector.tensor_scalar(spd_s[:, cols], p_cs[0:1, :nt],
                                        bso3_c, None, ALU.add)
                nc.scalar.activation(ctl_s[32:35, cols], p_cs[32:35, :nt],
                                     AF.Sigmoid, bias=bb3_c[:, k:k + 1])

            prev = None
            for t in range(len(TILES)):
                cur = stage_a(t)
                if prev is not None:
                    stage_b(prev)
                prev = cur
            stage_b(prev)

            nc.sync.dma_start(out=out4[0:3, :], in_=ctl_s[32:35, :])
            nc.sync.dma_start(out=out4[3:4, :], in_=spd_s[:])

    nc.compile()
    _cache["nc"] = nc
    return nc


# --------------------------------------------------------------------------
# Host-side routing / layout
# --------------------------------------------------------------------------

def _np16():
    if os.environ.get("KERNEL_DT", "float16") == "bfloat16":
        import ml_dtypes
        return ml_dtypes.bfloat16
    return np.float16


def _fm(w, dtype):
    """[K, ...] -> [128, K//128, ...] with contraction index f = o*128 + p."""
    ko = w.shape[0] // 128
    perm = (1, 0) + tuple(range(2, w.ndim + 1))
    return np.ascontiguousarray(
        w.reshape(ko, 128, *w.shape[1:]).transpose(*perm), dtype=dtype)


def _prep_weights(i):
    f32 = np.float32
    f16 = _np16()

    def a(x):
        return np.asarray(x, dtype=f32)

    # f16 blob A: [wsi2 | wso1 | wso2 | wso3] along free dim
    wsi2 = _fm(a(i["Wsi2"]), f16).reshape(128, -1)            # 256
    wso1 = _fm(a(i["Wso1"]), f16).reshape(128, -1)            # 1280
    wso2 = _fm(a(i["Wso2"]), f16).reshape(128, -1)            # 512
    wso3 = _fm(a(i["Wso3"]), f16).reshape(128, -1)            # 2
    wa = np.concatenate([wsi2, wso1, wso2, wso3], axis=1)
    assert wa.shape == (128, 2050)

    # f16 blob B, per-branch-contiguous: for each k [wb1_k | wb2_k | wb3_k]
    wb1 = _fm(a(i["Wb1"]).transpose(1, 0, 2), f16)   # [128, 5, 6, 256]
    wb2 = _fm(a(i["Wb2"]).transpose(1, 0, 2), f16)   # [128, 2, 6, 256]
    wb3 = _fm(a(i["Wb3"]).transpose(1, 0, 2), f16)   # [128, 2, 6, 3]
    parts = []
    for k in range(NBRANCH):
        parts.append(wb1[:, :, k, :].reshape(128, -1))   # 1280
        parts.append(wb2[:, :, k, :].reshape(128, -1))   # 512
        parts.append(wb3[:, :, k, :].reshape(128, -1))   # 6
    wb = np.ascontiguousarray(np.concatenate(parts, axis=1))
    assert wb.shape == (128, NBRANCH * WB_K), wb.shape

    # f32 blob: per-partition bias/scale columns
    wc = np.zeros((128, 40), dtype=f32)
    wc[:, 0:2] = _fm(a(i["bsi1"]), f32)
    wc[:, 2] = a(i["bsi2"])
    wc[:, 3:5] = _fm(a(i["bso1"]), f32)
    wc[:, 5:7] = _fm(a(i["bso2"]), f32)
    wc[:, 7:9] = _fm(a(i["Wsi1"]).reshape(256), f32)
    wc[:, 9:21] = _fm(a(i["bb1"]).T, f32).reshape(128, 12)
    wc[:, 21:33] = _fm(a(i["bb2"]).T, f32).reshape(128, 12)
    wc[0, 33] = a(i["bso3"])[0]
    wc[32:35, 34:40] = a(i["bb3"]).T
    return {"wa": wa, "wb": wb, "wc": wc}


def _route(cmd):
    """Assign rows to (core, slot-position); slot k of every core holds only
    branch-k rows. Returns idx [NCORES, BPC], valid [NCORES, BPC], spill."""
    idx = np.zeros((NCORES, BPC), dtype=np.int64)
    valid = np.zeros((NCORES, BPC), dtype=bool)
    spill = []
    for k in range(NBRANCH):
        rows = np.flatnonzero(cmd == k)
        for c, part in enumerate(np.array_split(rows, NCORES)):
            if len(part) > CAP:
                spill.append(part[CAP:])
                part = part[:CAP]
            idx[c, k * CAP:k * CAP + len(part)] = part
            valid[c, k * CAP:k * CAP + len(part)] = True
    spill = np.concatenate(spill) if spill else np.zeros(0, dtype=np.int64)
    return idx, valid, spill


def _np_reference(i, rows):
    """Exact reference math in numpy for a subset of rows (spill fallback)."""
    f32 = np.float32
    E = np.asarray(i["embedding"], f32)[rows]
    S = np.asarray(i["speed"], f32)[rows]
    cmd = np.asarray(i["command"])[rows].astype(np.int64) - 1
    sp = np.maximum(S @ np.asarray(i["Wsi1"], f32) + np.asarray(i["bsi1"], f32), 0)
    sp = sp @ np.asarray(i["Wsi2"], f32) + np.asarray(i["bsi2"], f32)
    emb = np.concatenate([E, sp], axis=1)
    hs = np.maximum(emb @ np.asarray(i["Wso1"], f32) + np.asarray(i["bso1"], f32), 0)
    hs = np.maximum(hs @ np.asarray(i["Wso2"], f32) + np.asarray(i["bso2"], f32), 0)
    spd = hs @ np.asarray(i["Wso3"], f32) + np.asarray(i["bso3"], f32)
    ctrl = np.zeros((len(rows), 3), f32)
    for k in range(NBRANCH):
        m = cmd == k
        if not m.any():
            continue
        h = np.maximum(emb[m] @ np.asarray(i["Wb1"], f32)[k]
                       + np.asarray(i["bb1"], f32)[k], 0)
        h = np.maximum(h @ np.asarray(i["Wb2"], f32)[k]
                       + np.asarray(i["bb2"], f32)[k], 0)
        z = h @ np.asarray(i["Wb3"], f32)[k] + np.asarray(i["bb3"], f32)[k]
        ctrl[m] = 1.0 / (1.0 + np.exp(-z))
    return ctrl, spd.astype(f32)


# --------------------------------------------------------------------------
# Entry point
# --------------------------------------------------------------------------

LAST_RESULTS = None  # BassKernelResults of the most recent device run


def kernel(embedding, speed, command, **weights):
    global LAST_RESULTS
    inputs = dict(weights)
    inputs.update(embedding=embedding, speed=speed, command=command)

    embedding = np.asarray(embedding, dtype=np.float32)
    speed = np.asarray(speed, dtype=np.float32)
    command_np = np.asarray(command)

    if embedding.shape != (B, D_EMB):
        # Unexpected problem size: fall back to exact host computation.
        ctrl, spd = _np_reference(inputs, np.arange(embedding.shape[0]))
        return ctrl, spd

    cmd = command_np.astype(np.int64) - 1
    idx, valid, spill = _route(cmd)

    w = _prep_weights(inputs)
    f16 = _np16()

    in_maps = []
    for c in range(NCORES):
        rows = idx[c]
        emb_t = embedding[rows].T.astype(f16)                  # [512, BPC]
        emb_t = emb_t.reshape(4, 128, BPC).transpose(1, 0, 2)  # [128, 4, BPC]
        spd_b = speed[rows, 0].astype(f16)                     # [BPC]
        xt = np.empty(XT_ELEMS, dtype=f16)
        for (k, off, nt, eoff) in TILES:
            c0 = k * CAP + off
            blk = xt[eoff:eoff + 128 * 5 * nt].reshape(128, 5, nt)
            blk[:, 0:4, :] = emb_t[:, :, c0:c0 + nt]
            blk[:, 4, :] = spd_b[c0:c0 + nt][None, :]
        m = {"xt": xt}
        m.update(w)
        in_maps.append(m)

    from concourse.bass_utils import run_bass_kernel_spmd

    nc = _build_nc()
    res = run_bass_kernel_spmd(
        nc, in_maps, core_ids=list(range(NCORES)),
        trace=bool(int(os.environ.get("KERNEL_TRACE", "0"))),
    )
    LAST_RESULTS = res

    control = np.zeros((B, 3), dtype=np.float32)
    speed_pred = np.zeros((B, 1), dtype=np.float32)
    for c in range(NCORES):
        o4 = np.asarray(res.results[c]["out4"])
        v = valid[c]
        rows = idx[c][v]
        control[rows] = o4[0:3, v].T
        speed_pred[rows, 0] = o4[3, v]

    if len(spill):
        ctrl_sp, spd_sp = _np_reference(inputs, spill)
        control[spill] = ctrl_sp
        speed_pred[spill] = spd_sp

    return control, speed_pred


# revision 17
# speedup vs baseline: 1.2160x; 1.0168x over previous
"""Trainium2 Bass kernel for nn_CILRSModel (moe_routing).

Strategy:
  - Host-side MoE routing: rows are bucketed by `command` (6 branches) and
    distributed evenly over 8 cores. Each core gets a fixed [6 x CAP] row
    layout so the SPMD kernel statically knows which branch weights apply
    to which batch tile (no on-device routing control flow).
  - Host-side transpose: the embedding is shipped feature-major (partition =
    feature), per batch-tile contiguous, so every matmul operand already has
    the contraction dim on SBUF partitions and every x load is one fast DMA.
    The speed scalar is shipped broadcast across all 128 partitions so the
    speed-MLP's first (K=1) layer runs on the Scalar engine.
  - On device everything is feature-major fp16 (PSUM accumulates fp32).
    Two-stage software pipeline: tile t+1's first-layer matmuls are emitted
    before tile t's second/third layers so eviction latency hides behind
    independent PE work.
  - The two tiny-M output matmuls (control M=3, speed M=1) are packed into
    distinct PE column groups via tile_position so they run concurrently.
  - Outputs return feature-major as out4 = [4, rows] and are scattered back
    to the original row order on host.
"""

import os
import sys

import numpy as np

_TRN_REPO = "/opt/trn_rl_repo"
if _TRN_REPO not in sys.path:
    sys.path.insert(0, _TRN_REPO)

# Problem constants (hardcoded per harness contract)
B = 65536
D_EMB = 512
D_LAT = 128
H = 256
NBRANCH = 6
D_IN = D_EMB + D_LAT  # 640
NCORES = 8
CAP = 1408            # per-core per-branch row capacity (actual max ~1389)
BPC = NBRANCH * CAP   # 8448 rows per core
SLOT_TILES = [(0, 512), (512, 512), (1024, 384)]
WB_K = 1798           # per-branch weight-blob width: 5*256 + 2*256 + 2*3

# tile table: (branch k, col offset, width, xt element offset)
TILES = []
_eoff = 0
for _k in range(NBRANCH):
    for _off, _nt in SLOT_TILES:
        TILES.append((_k, _off, _nt, _eoff))
        _eoff += 128 * 5 * _nt
XT_ELEMS = _eoff

_cache = {}


# --------------------------------------------------------------------------
# Device kernel
# --------------------------------------------------------------------------

def _build_nc():
    if "nc" in _cache:
        return _cache["nc"]

    import concourse.mybir as mybir
    import concourse.tile as tile
    from concourse import bacc

    f32 = mybir.dt.float32
    f16 = getattr(mybir.dt, os.environ.get("KERNEL_DT", "float16"))
    AF = mybir.ActivationFunctionType
    ALU = mybir.AluOpType

    nc = bacc.Bacc("TRN2", target_bir_lowering=False, debug=False,
                   num_devices=NCORES)

    xt = nc.dram_tensor("xt", [XT_ELEMS], f16, kind="ExternalInput")[:]
    wa = nc.dram_tensor("wa", [128, 2050], f16, kind="ExternalInput")[:]
    wb = nc.dram_tensor("wb", [128, NBRANCH * WB_K], f16, kind="ExternalInput")[:]
    wc = nc.dram_tensor("wc", [128, 40], f32, kind="ExternalInput")[:]
    out4 = nc.dram_tensor("out4", [4, BPC], f32, kind="ExternalOutput")[:]

    with tile.TileContext(nc) as tc:
        with (
            tc.tile_pool(name="wpool", bufs=1) as wpool,
            tc.tile_pool(name="xpool", bufs=4) as xpool,
            tc.tile_pool(name="hpool", bufs=2) as hpool,
            tc.tile_pool(name="opool", bufs=1) as opool,
            tc.tile_pool(name="pmm", bufs=3, space="PSUM") as pmm,
            tc.tile_pool(name="pm1", bufs=2, space="PSUM") as pm1,
        ):
            wc_s = wpool.tile([128, 40], f32, tag="wc")
            nc.sync.dma_start(out=wc_s[:], in_=wc)
            wa_s = wpool.tile([128, 2050], f16, tag="wa")
            nc.sync.dma_start(out=wa_s[:], in_=wa)
            wb_s = wpool.tile([128, NBRANCH * WB_K], f16, tag="wb")
            # branch 0 weights up front; later branches prefetched in-loop
            nc.sync.dma_start(out=wb_s[:, 0:WB_K], in_=wb[:, 0:WB_K])

            # f16 blob A slices
            def wsi2_sl(o):
                return wa_s[:, o * 128:(o + 1) * 128]

            def wso1_sl(o, j):
                c = 256 + o * 256 + j * 128
                return wa_s[:, c:c + 128]

            def wso2_sl(o, j):
                c = 1536 + o * 256 + j * 128
                return wa_s[:, c:c + 128]

            def wso3_sl(o):
                return wa_s[:, 2048 + o:2049 + o]

            # f16 blob B slices (per-branch-contiguous)
            def wb1_sl(o, k, j):
                c = k * WB_K + o * 256 + j * 128
                return wb_s[:, c:c + 128]

            def wb2_sl(o, k, j):
                c = k * WB_K + 1280 + o * 256 + j * 128
                return wb_s[:, c:c + 128]

            def wb3_sl(o, k):
                c = k * WB_K + 1792 + o * 3
                return wb_s[:, c:c + 3]

            # f32 blob slices (per-partition bias/scale columns)
            def bsi1_c(j):
                return wc_s[:, j:j + 1]

            bsi2_c = wc_s[:, 2:3]

            def bso1_c(j):
                return wc_s[:, 3 + j:4 + j]

            def bso2_c(j):
                return wc_s[:, 5 + j:6 + j]

            def wsi1_c(j):
                return wc_s[:, 7 + j:8 + j]

            def bb1_c(j, k):
                c = 9 + j * 6 + k
                return wc_s[:, c:c + 1]

            def bb2_c(j, k):
                c = 21 + j * 6 + k
                return wc_s[:, c:c + 1]

            bso3_c = wc_s[0:1, 33:34]
            bb3_c = wc_s[32:35, 34:40]   # bb3 lives at partitions 32:35

            ctl_s = opool.tile([35, BPC], f32, tag="octl")  # rows 32:35 used
            spd_s = opool.tile([1, BPC], f32, tag="ospd")

            def evict_relu(dst, src, bias_ap, on_act):
                if on_act:
                    nc.scalar.activation(dst, src, AF.Relu, bias=bias_ap)
                else:
                    nc.vector.tensor_scalar(dst, src, bias_ap, 0.0, ALU.add, ALU.max)

            def sp_chain(t):
                """x load + speed-MLP (hsp on ACT, latent matmuls + evict)."""
                k, off, nt, eoff = TILES[t]

                x_s = xpool.tile([128, 5, nt], f16, tag=f"x{nt}")
                src = xt[eoff:eoff + 128 * 5 * nt].rearrange(
                    "(p o b) -> p o b", p=128, o=5)
                nc.sync.dma_start(out=x_s[:], in_=src)

                # prefetch the next branch's weights at each slot start
                if off == 0 and k + 1 < NBRANCH:
                    c = (k + 1) * WB_K
                    nc.sync.dma_start(out=wb_s[:, c:c + WB_K],
                                      in_=wb[:, c:c + WB_K])

                # speed-MLP layer 1 on ScalarE (per-partition scale+bias+relu)
                hsp = hpool.tile([128, 2, nt], f16, tag=f"hsp{nt}")
                for j in range(2):
                    nc.scalar.activation(hsp[:, j, :], x_s[:, 4, :], AF.Relu,
                                         bias=bsi1_c(j), scale=wsi1_c(j))
                return (x_s, hsp, nt)

            def sp_finish(st):
                """speed-MLP layer 2 -> latent overwrites x_s[:,4,:]."""
                x_s, hsp, nt = st
                p_sp = pm1.tile([128, 512], f32, tag="pm1")
                nc.tensor.matmul(p_sp[:, :nt], wsi2_sl(0), hsp[:, 0, :],
                                 start=True, stop=False)
                nc.tensor.matmul(p_sp[:, :nt], wsi2_sl(1), hsp[:, 1, :],
                                 start=False, stop=True)
                nc.vector.tensor_scalar(x_s[:, 4, :], p_sp[:, :nt], bsi2_c,
                                        None, ALU.add)

            def layer1(t, st):
                """Layer-1 of both heads + h1/g1 evictions."""
                x_s, hsp, nt = st
                k, off, _, _ = TILES[t]

                p1 = pmm.tile([128, 2, 512], f32, tag="pmm")
                for j in range(2):
                    for o in range(4):
                        nc.tensor.matmul(p1[:, j, :nt], wso1_sl(o, j),
                                         x_s[:, o, :],
                                         start=(o == 0), stop=False)
                q1 = pmm.tile([128, 2, 512], f32, tag="pmm")
                for j in range(2):
                    for o in range(4):
                        nc.tensor.matmul(q1[:, j, :nt], wb1_sl(o, k, j),
                                         x_s[:, o, :],
                                         start=(o == 0), stop=False)
                # latent subtile finishers (x_s[:,4,:] written last cycle)
                for j in range(2):
                    nc.tensor.matmul(p1[:, j, :nt], wso1_sl(4, j),
                                     x_s[:, 4, :], start=False, stop=True)
                for j in range(2):
                    nc.tensor.matmul(q1[:, j, :nt], wb1_sl(4, k, j),
                                     x_s[:, 4, :], start=False, stop=True)

                h1 = hpool.tile([128, 2, nt], f16, tag=f"h1{nt}")
                evict_relu(h1[:, 0, :], p1[:, 0, :nt], bso1_c(0), True)
                evict_relu(h1[:, 1, :], p1[:, 1, :nt], bso1_c(1), False)
                g1 = hpool.tile([128, 2, nt], f16, tag=f"g1{nt}")
                evict_relu(g1[:, 0, :], q1[:, 0, :nt], bb1_c(0, k), True)
                evict_relu(g1[:, 1, :], q1[:, 1, :nt], bb1_c(1, k), False)
                return (k, off, nt, h1, g1)

            def layer23(st):
                """Layer-2 of both heads, output layer, final evictions."""
                k, off, nt, h1, g1 = st
                c0 = k * CAP + off
                cols = slice(c0, c0 + nt)

                p2 = pmm.tile([128, 2, 512], f32, tag="pmm")
                for j in range(2):
                    for o in range(2):
                        nc.tensor.matmul(p2[:, j, :nt], wso2_sl(o, j),
                                         h1[:, o, :],
                                         start=(o == 0), stop=(o == 1))
                q2 = pmm.tile([128, 2, 512], f32, tag="pmm")
                for j in range(2):
                    for o in range(2):
                        nc.tensor.matmul(q2[:, j, :nt], wb2_sl(o, k, j),
                                         g1[:, o, :],
                                         start=(o == 0), stop=(o == 1))

                h2 = hpool.tile([128, 2, nt], f16, tag=f"h2{nt}")
                evict_relu(h2[:, 0, :], p2[:, 0, :nt], bso2_c(0), True)
                evict_relu(h2[:, 1, :], p2[:, 1, :nt], bso2_c(1), False)
                g2 = hpool.tile([128, 2, nt], f16, tag=f"g2{nt}")
                evict_relu(g2[:, 0, :], q2[:, 0, :nt], bb2_c(0, k), False)
                evict_relu(g2[:, 1, :], q2[:, 1, :nt], bb2_c(1, k), False)

                # outputs: spd (M=1) in col group 0, ctrl (M=3) in col group 1
                p_cs = pm1.tile([64, 512], f32, tag="pm1")
                nc.tensor.matmul(p_cs[0:1, :nt], wso3_sl(0), h2[:, 0, :],
                                 start=True, stop=False, tile_position=(0, 0))
                nc.tensor.matmul(p_cs[0:1, :nt], wso3_sl(1), h2[:, 1, :],
                                 start=False, stop=True, tile_position=(0, 0))
                nc.tensor.matmul(p_cs[32:35, :nt], wb3_sl(0, k), g2[:, 0, :],
                                 start=True, stop=False, tile_position=(0, 32))
                nc.tensor.matmul(p_cs[32:35, :nt], wb3_sl(1, k), g2[:, 1, :],
                                 start=False, stop=True, tile_position=(0, 32))
                nc.vector.tensor_scalar(spd_s[:, cols], p_cs[0:1, :nt],
                                        bso3_c, None, ALU.add)
                nc.scalar.activation(ctl_s[32:35, cols], p_cs[32:35, :nt],
                                     AF.Sigmoid, bias=bb3_c[:, k:k + 1])
                # stream results out once a branch slot's last tile is done
                if off == 1024:
                    kcols = slice(k * CAP, (k + 1) * CAP)
                    nc.sync.dma_start(out=out4[0:3, kcols],
                                      in_=ctl_s[32:35, kcols])
                    nc.sync.dma_start(out=out4[3:4, kcols],
                                      in_=spd_s[:, kcols])

            # 3-stage pipeline: sp-chain(t) | layer1(t-1) | layer23(t-2)
            NT_ = len(TILES)
            sp_states = {}
            l1_states = {}
            for t in range(NT_ + 2):
                if t < NT_:
                    sp_states[t] = sp_chain(t)
                if t - 1 >= 0 and t - 1 < NT_:
                    l1_states[t - 1] = layer1(t - 1, sp_states[t - 1])
                if t - 2 >= 0:
                    layer23(l1_states.pop(t - 2))
                if t < NT_:
                    sp_finish(sp_states[t])
                if t - 1 >= 0 and t - 1 < NT_:
                    sp_states.pop(t - 1)

    nc.compile()
    _cache["nc"] = nc
    return nc


# --------------------------------------------------------------------------
# Host-side routing / layout
# --------------------------------------------------------------------------

def _np16():
    if os.environ.get("KERNEL_DT", "float16") == "bfloat16":
        import ml_dtypes
        return ml_dtypes.bfloat16
    return np.float16


def _fm(w, dtype):
    """[K, ...] -> [128, K//128, ...] with contraction index f = o*128 + p."""
    ko = w.shape[0] // 128
    perm = (1, 0) + tuple(range(2, w.ndim + 1))
    return np.ascontiguousarray(
        w.reshape(ko, 128, *w.shape[1:]).transpose(*perm), dtype=dtype)


def _prep_weights(i):
    f32 = np.float32
    f16 = _np16()

    def a(x):
        return np.asarray(x, dtype=f32)

    # f16 blob A: [wsi2 | wso1 | wso2 | wso3] along free dim
    wsi2 = _fm(a(i["Wsi2"]), f16).reshape(128, -1)            # 256
    wso1 = _fm(a(i["Wso1"]), f16).reshape(128, -1)            # 1280
    wso2 = _fm(a(i["Wso2"]), f16).reshape(128, -1)            # 512
    wso3 = _fm(a(i["Wso3"]), f16).reshape(128, -1)            # 2
    wa = np.concatenate([wsi2, wso1, wso2, wso3], axis=1)
    assert wa.shape == (128, 2050)

    # f16 blob B, per-branch-contiguous: for each k [wb1_k | wb2_k | wb3_k]
    wb1 = _fm(a(i["Wb1"]).transpose(1, 0, 2), f16)   # [128, 5, 6, 256]
    wb2 = _fm(a(i["Wb2"]).transpose(1, 0, 2), f16)   # [128, 2, 6, 256]
    wb3 = _fm(a(i["Wb3"]).transpose(1, 0, 2), f16)   # [128, 2, 6, 3]
    parts = []
    for k in range(NBRANCH):
        parts.append(wb1[:, :, k, :].reshape(128, -1))   # 1280
        parts.append(wb2[:, :, k, :].reshape(128, -1))   # 512
        parts.append(wb3[:, :, k, :].reshape(128, -1))   # 6
    wb = np.ascontiguousarray(np.concatenate(parts, axis=1))
    assert wb.shape == (128, NBRANCH * WB_K), wb.shape

    # f32 blob: per-partition bias/scale columns
    wc = np.zeros((128, 40), dtype=f32)
    wc[:, 0:2] = _fm(a(i["bsi1"]), f32)
    wc[:, 2] = a(i["bsi2"])
    wc[:, 3:5] = _fm(a(i["bso1"]), f32)
    wc[:, 5:7] = _fm(a(i["bso2"]), f32)
    wc[:, 7:9] = _fm(a(i["Wsi1"]).reshape(256), f32)
    wc[:, 9:21] = _fm(a(i["bb1"]).T, f32).reshape(128, 12)
    wc[:, 21:33] = _fm(a(i["bb2"]).T, f32).reshape(128, 12)
    wc[0, 33] = a(i["bso3"])[0]
    wc[32:35, 34:40] = a(i["bb3"]).T
    return {"wa": wa, "wb": wb, "wc": wc}


def _route(cmd):
    """Assign rows to (core, slot-position); slot k of every core holds only
    branch-k rows. Returns idx [NCORES, BPC], valid [NCORES, BPC], spill."""
    idx = np.zeros((NCORES, BPC), dtype=np.int64)
    valid = np.zeros((NCORES, BPC), dtype=bool)
    spill = []
    for k in range(NBRANCH):
        rows = np.flatnonzero(cmd == k)
        for c, part in enumerate(np.array_split(rows, NCORES)):
            if len(part) > CAP:
                spill.append(part[CAP:])
                part = part[:CAP]
            idx[c, k * CAP:k * CAP + len(part)] = part
            valid[c, k * CAP:k * CAP + len(part)] = True
    spill = np.concatenate(spill) if spill else np.zeros(0, dtype=np.int64)
    return idx, valid, spill


def _np_reference(i, rows):
    """Exact reference math in numpy for a subset of rows (spill fallback)."""
    f32 = np.float32
    E = np.asarray(i["embedding"], f32)[rows]
    S = np.asarray(i["speed"], f32)[rows]
    cmd = np.asarray(i["command"])[rows].astype(np.int64) - 1
    sp = np.maximum(S @ np.asarray(i["Wsi1"], f32) + np.asarray(i["bsi1"], f32), 0)
    sp = sp @ np.asarray(i["Wsi2"], f32) + np.asarray(i["bsi2"], f32)
    emb = np.concatenate([E, sp], axis=1)
    hs = np.maximum(emb @ np.asarray(i["Wso1"], f32) + np.asarray(i["bso1"], f32), 0)
    hs = np.maximum(hs @ np.asarray(i["Wso2"], f32) + np.asarray(i["bso2"], f32), 0)
    spd = hs @ np.asarray(i["Wso3"], f32) + np.asarray(i["bso3"], f32)
    ctrl = np.zeros((len(rows), 3), f32)
    for k in range(NBRANCH):
        m = cmd == k
        if not m.any():
            continue
        h = np.maximum(emb[m] @ np.asarray(i["Wb1"], f32)[k]
                       + np.asarray(i["bb1"], f32)[k], 0)
        h = np.maximum(h @ np.asarray(i["Wb2"], f32)[k]
                       + np.asarray(i["bb2"], f32)[k], 0)
        z = h @ np.asarray(i["Wb3"], f32)[k] + np.asarray(i["bb3"], f32)[k]
        ctrl[m] = 1.0 / (1.0 + np.exp(-z))
    return ctrl, spd.astype(f32)


# --------------------------------------------------------------------------
# Entry point
# --------------------------------------------------------------------------

LAST_RESULTS = None  # BassKernelResults of the most recent device run


def kernel(embedding, speed, command, **weights):
    global LAST_RESULTS
    inputs = dict(weights)
    inputs.update(embedding=embedding, speed=speed, command=command)

    embedding = np.asarray(embedding, dtype=np.float32)
    speed = np.asarray(speed, dtype=np.float32)
    command_np = np.asarray(command)

    if embedding.shape != (B, D_EMB):
        # Unexpected problem size: fall back to exact host computation.
        ctrl, spd = _np_reference(inputs, np.arange(embedding.shape[0]))
        return ctrl, spd

    cmd = command_np.astype(np.int64) - 1
    idx, valid, spill = _route(cmd)

    w = _prep_weights(inputs)
    f16 = _np16()

    in_maps = []
    for c in range(NCORES):
        rows = idx[c]
        emb_t = embedding[rows].T.astype(f16)                  # [512, BPC]
        emb_t = emb_t.reshape(4, 128, BPC).transpose(1, 0, 2)  # [128, 4, BPC]
        spd_b = speed[rows, 0].astype(f16)                     # [BPC]
        xt = np.empty(XT_ELEMS, dtype=f16)
        for (k, off, nt, eoff) in TILES:
            c0 = k * CAP + off
            blk = xt[eoff:eoff + 128 * 5 * nt].reshape(128, 5, nt)
            blk[:, 0:4, :] = emb_t[:, :, c0:c0 + nt]
            blk[:, 4, :] = spd_b[c0:c0 + nt][None, :]
        m = {"xt": xt}
        m.update(w)
        in_maps.append(m)

    from concourse.bass_utils import run_bass_kernel_spmd

    nc = _build_nc()
    res = run_bass_kernel_spmd(
        nc, in_maps, core_ids=list(range(NCORES)),
        trace=bool(int(os.environ.get("KERNEL_TRACE", "0"))),
    )
    LAST_RESULTS = res

    control = np.zeros((B, 3), dtype=np.float32)
    speed_pred = np.zeros((B, 1), dtype=np.float32)
    for c in range(NCORES):
        o4 = np.asarray(res.results[c]["out4"])
        v = valid[c]
        rows = idx[c][v]
        control[rows] = o4[0:3, v].T
        speed_pred[rows, 0] = o4[3, v]

    if len(spill):
        ctrl_sp, spd_sp = _np_reference(inputs, spill)
        control[spill] = ctrl_sp
        speed_pred[spill] = spd_sp

    return control, speed_pred


# revision 18
# speedup vs baseline: 1.2324x; 1.0135x over previous
"""Trainium2 Bass kernel for nn_CILRSModel (moe_routing).

Strategy:
  - Host-side MoE routing: rows are bucketed by `command` (6 branches) and
    distributed evenly over 8 cores. Each core gets a fixed [6 x CAP] row
    layout so the SPMD kernel statically knows which branch weights apply
    to which batch tile (no on-device routing control flow).
  - Host-side transpose: the embedding is shipped feature-major (partition =
    feature), per batch-tile contiguous, so every matmul operand already has
    the contraction dim on SBUF partitions and every x load is one fast DMA.
    The speed scalar is shipped broadcast across all 128 partitions so the
    speed-MLP's first (K=1) layer runs on the Scalar engine.
  - On device everything is feature-major fp16 (PSUM accumulates fp32).
    Two-stage software pipeline: tile t+1's first-layer matmuls are emitted
    before tile t's second/third layers so eviction latency hides behind
    independent PE work.
  - The two tiny-M output matmuls (control M=3, speed M=1) are packed into
    distinct PE column groups via tile_position so they run concurrently.
  - Outputs return feature-major as out4 = [4, rows] and are scattered back
    to the original row order on host.
"""

import os
import sys

import numpy as np

_TRN_REPO = "/opt/trn_rl_repo"
if _TRN_REPO not in sys.path:
    sys.path.insert(0, _TRN_REPO)

# Problem constants (hardcoded per harness contract)
B = 65536
D_EMB = 512
D_LAT = 128
H = 256
NBRANCH = 6
D_IN = D_EMB + D_LAT  # 640
NCORES = 8
CAP = 1408            # per-core per-branch row capacity (actual max ~1389)
BPC = NBRANCH * CAP   # 8448 rows per core
SLOT_TILES = [(0, 512), (512, 512), (1024, 384)]
WB_K = 1798           # per-branch weight-blob width: 5*256 + 2*256 + 2*3

# tile table: (branch k, col offset, width, xt element offset)
TILES = []
_eoff = 0
for _k in range(NBRANCH):
    for _off, _nt in SLOT_TILES:
        TILES.append((_k, _off, _nt, _eoff))
        _eoff += 128 * 5 * _nt
XT_ELEMS = _eoff

_cache = {}


# --------------------------------------------------------------------------
# Device kernel
# --------------------------------------------------------------------------

def _build_nc():
    if "nc" in _cache:
        return _cache["nc"]

    import concourse.mybir as mybir
    import concourse.tile as tile
    from concourse import bacc

    f32 = mybir.dt.float32
    f16 = getattr(mybir.dt, os.environ.get("KERNEL_DT", "float16"))
    AF = mybir.ActivationFunctionType
    ALU = mybir.AluOpType

    nc = bacc.Bacc("TRN2", target_bir_lowering=False, debug=False,
                   num_devices=NCORES)

    xt = nc.dram_tensor("xt", [XT_ELEMS], f16, kind="ExternalInput")[:]
    wa = nc.dram_tensor("wa", [128, 2050], f16, kind="ExternalInput")[:]
    wb = nc.dram_tensor("wb", [128, NBRANCH * WB_K], f16, kind="ExternalInput")[:]
    wc = nc.dram_tensor("wc", [128, 40], f32, kind="ExternalInput")[:]
    out4 = nc.dram_tensor("out4", [4, BPC], f32, kind="ExternalOutput")[:]

    with tile.TileContext(nc) as tc:
        with (
            tc.tile_pool(name="wpool", bufs=1) as wpool,
            tc.tile_pool(name="xpool", bufs=4) as xpool,
            tc.tile_pool(name="hpool", bufs=2) as hpool,
            tc.tile_pool(name="opool", bufs=1) as opool,
            tc.tile_pool(name="pmm", bufs=3, space="PSUM") as pmm,
            tc.tile_pool(name="pm1", bufs=2, space="PSUM") as pm1,
        ):
            wc_s = wpool.tile([128, 40], f32, tag="wc")
            nc.sync.dma_start(out=wc_s[:], in_=wc)
            wa_s = wpool.tile([128, 2050], f16, tag="wa")
            nc.sync.dma_start(out=wa_s[:], in_=wa)
            wb_s = wpool.tile([128, NBRANCH * WB_K], f16, tag="wb")
            # branch 0 weights up front; later branches prefetched in-loop
            nc.sync.dma_start(out=wb_s[:, 0:WB_K], in_=wb[:, 0:WB_K])

            # f16 blob A slices
            def wsi2_sl(o):
                return wa_s[:, o * 128:(o + 1) * 128]

            def wso1_sl(o, j):
                c = 256 + o * 256 + j * 128
                return wa_s[:, c:c + 128]

            def wso2_sl(o, j):
                c = 1536 + o * 256 + j * 128
                return wa_s[:, c:c + 128]

            def wso3_sl(o):
                return wa_s[:, 2048 + o:2049 + o]

            # f16 blob B slices (per-branch-contiguous)
            def wb1_sl(o, k, j):
                c = k * WB_K + o * 256 + j * 128
                return wb_s[:, c:c + 128]

            def wb2_sl(o, k, j):
                c = k * WB_K + 1280 + o * 256 + j * 128
                return wb_s[:, c:c + 128]

            def wb3_sl(o, k):
                c = k * WB_K + 1792 + o * 3
                return wb_s[:, c:c + 3]

            # f32 blob slices (per-partition bias/scale columns)
            def bsi1_c(j):
                return wc_s[:, j:j + 1]

            bsi2_c = wc_s[:, 2:3]

            def bso1_c(j):
                return wc_s[:, 3 + j:4 + j]

            def bso2_c(j):
                return wc_s[:, 5 + j:6 + j]

            def wsi1_c(j):
                return wc_s[:, 7 + j:8 + j]

            def bb1_c(j, k):
                c = 9 + j * 6 + k
                return wc_s[:, c:c + 1]

            def bb2_c(j, k):
                c = 21 + j * 6 + k
                return wc_s[:, c:c + 1]

            bso3_c = wc_s[0:1, 33:34]
            bb3_c = wc_s[32:35, 34:40]   # bb3 lives at partitions 32:35

            ctl_s = opool.tile([35, BPC], f32, tag="octl")  # rows 32:35 used
            spd_s = opool.tile([1, BPC], f32, tag="ospd")

            # warm-up: pull the ACT table load forward (gated only on the tiny
            # wc blob) and keep the PE busy/HAM-warm before real tiles arrive
            warm = wpool.tile([1, 8], f32, tag="warm")
            nc.scalar.activation(warm[:], wc_s[0:1, 0:8], AF.Relu,
                                 bias=bsi1_c(0)[0:1])
            pwarm = pm1.tile([128, 512], f32, tag="pm1")
            for _ in range(8):
                nc.tensor.matmul(pwarm[:], wa_s[:, 0:128], wa_s[:, 512:1024],
                                 start=True, stop=True)

            def evict_relu(dst, src, bias_ap, on_act):
                if on_act:
                    nc.scalar.activation(dst, src, AF.Relu, bias=bias_ap)
                else:
                    nc.vector.tensor_scalar(dst, src, bias_ap, 0.0, ALU.add, ALU.max)

            def sp_load(t):
                """x tile DMA + branch-weight prefetch."""
                k, off, nt, eoff = TILES[t]
                x_s = xpool.tile([128, 5, nt], f16, tag=f"x{nt}")
                src = xt[eoff:eoff + 128 * 5 * nt].rearrange(
                    "(p o b) -> p o b", p=128, o=5)
                nc.sync.dma_start(out=x_s[:], in_=src)
                if off == 0 and k + 1 < NBRANCH:
                    c = (k + 1) * WB_K
                    nc.sync.dma_start(out=wb_s[:, c:c + WB_K],
                                      in_=wb[:, c:c + WB_K])
                return (x_s, nt)

            def sp_act(t, ld):
                """speed-MLP layer 1 on ScalarE (scale+bias+relu)."""
                x_s, nt = ld
                hsp = hpool.tile([128, 2, nt], f16, tag=f"hsp{nt}")
                for j in range(2):
                    nc.scalar.activation(hsp[:, j, :], x_s[:, 4, :], AF.Relu,
                                         bias=bsi1_c(j), scale=wsi1_c(j))
                return (x_s, hsp, nt)

            def sp_finish(st):
                """speed-MLP layer 2 -> latent overwrites x_s[:,4,:]."""
                x_s, hsp, nt = st
                p_sp = pm1.tile([128, 512], f32, tag="pm1")
                nc.tensor.matmul(p_sp[:, :nt], wsi2_sl(0), hsp[:, 0, :],
                                 start=True, stop=False)
                nc.tensor.matmul(p_sp[:, :nt], wsi2_sl(1), hsp[:, 1, :],
                                 start=False, stop=True)
                nc.vector.tensor_scalar(x_s[:, 4, :], p_sp[:, :nt], bsi2_c,
                                        None, ALU.add)

            def layer1(t, st):
                """Layer-1 of both heads + h1/g1 evictions."""
                x_s, hsp, nt = st
                k, off, _, _ = TILES[t]

                p1 = pmm.tile([128, 2, 512], f32, tag="pmm")
                for j in range(2):
                    for o in range(4):
                        nc.tensor.matmul(p1[:, j, :nt], wso1_sl(o, j),
                                         x_s[:, o, :],
                                         start=(o == 0), stop=False)
                q1 = pmm.tile([128, 2, 512], f32, tag="pmm")
                for j in range(2):
                    for o in range(4):
                        nc.tensor.matmul(q1[:, j, :nt], wb1_sl(o, k, j),
                                         x_s[:, o, :],
                                         start=(o == 0), stop=False)
                # latent subtile finishers (x_s[:,4,:] written last cycle)
                for j in range(2):
                    nc.tensor.matmul(p1[:, j, :nt], wso1_sl(4, j),
                                     x_s[:, 4, :], start=False, stop=True)
                for j in range(2):
                    nc.tensor.matmul(q1[:, j, :nt], wb1_sl(4, k, j),
                                     x_s[:, 4, :], start=False, stop=True)

                h1 = hpool.tile([128, 2, nt], f16, tag=f"h1{nt}")
                evict_relu(h1[:, 0, :], p1[:, 0, :nt], bso1_c(0), True)
                evict_relu(h1[:, 1, :], p1[:, 1, :nt], bso1_c(1), False)
                g1 = hpool.tile([128, 2, nt], f16, tag=f"g1{nt}")
                evict_relu(g1[:, 0, :], q1[:, 0, :nt], bb1_c(0, k), False)
                evict_relu(g1[:, 1, :], q1[:, 1, :nt], bb1_c(1, k), False)
                return (k, off, nt, h1, g1)

            def layer23(st):
                """Layer-2 of both heads, output layer, final evictions."""
                k, off, nt, h1, g1 = st
                c0 = k * CAP + off
                cols = slice(c0, c0 + nt)

                p2 = pmm.tile([128, 2, 512], f32, tag="pmm")
                for j in range(2):
                    for o in range(2):
                        nc.tensor.matmul(p2[:, j, :nt], wso2_sl(o, j),
                                         h1[:, o, :],
                                         start=(o == 0), stop=(o == 1))
                q2 = pmm.tile([128, 2, 512], f32, tag="pmm")
                for j in range(2):
                    for o in range(2):
                        nc.tensor.matmul(q2[:, j, :nt], wb2_sl(o, k, j),
                                         g1[:, o, :],
                                         start=(o == 0), stop=(o == 1))

                h2 = hpool.tile([128, 2, nt], f16, tag=f"h2{nt}")
                evict_relu(h2[:, 0, :], p2[:, 0, :nt], bso2_c(0), True)
                evict_relu(h2[:, 1, :], p2[:, 1, :nt], bso2_c(1), False)
                g2 = hpool.tile([128, 2, nt], f16, tag=f"g2{nt}")
                evict_relu(g2[:, 0, :], q2[:, 0, :nt], bb2_c(0, k), False)
                evict_relu(g2[:, 1, :], q2[:, 1, :nt], bb2_c(1, k), False)

                # outputs: spd (M=1) in col group 0, ctrl (M=3) in col group 1
                p_cs = pm1.tile([64, 512], f32, tag="pm1")
                nc.tensor.matmul(p_cs[0:1, :nt], wso3_sl(0), h2[:, 0, :],
                                 start=True, stop=False, tile_position=(0, 0))
                nc.tensor.matmul(p_cs[0:1, :nt], wso3_sl(1), h2[:, 1, :],
                                 start=False, stop=True, tile_position=(0, 0))
                nc.tensor.matmul(p_cs[32:35, :nt], wb3_sl(0, k), g2[:, 0, :],
                                 start=True, stop=False, tile_position=(0, 32))
                nc.tensor.matmul(p_cs[32:35, :nt], wb3_sl(1, k), g2[:, 1, :],
                                 start=False, stop=True, tile_position=(0, 32))
                nc.scalar.activation(spd_s[:, cols], p_cs[0:1, :nt],
                                     AF.Identity, bias=bso3_c)
                nc.scalar.activation(ctl_s[32:35, cols], p_cs[32:35, :nt],
                                     AF.Sigmoid, bias=bb3_c[:, k:k + 1])
                # stream results out once a branch slot's last tile is done
                if off == 1024:
                    kcols = slice(k * CAP, (k + 1) * CAP)
                    nc.sync.dma_start(out=out4[0:3, kcols],
                                      in_=ctl_s[32:35, kcols])
                    nc.sync.dma_start(out=out4[3:4, kcols],
                                      in_=spd_s[:, kcols])

            # 3-stage pipeline with one-cycle DMA look-ahead:
            # load(t+1) | layer1(t-1) | layer23(t-2) | sp(t)
            NT_ = len(TILES)
            loads = {0: sp_load(0)}
            sp_states = {}
            l1_states = {}
            for t in range(NT_ + 2):
                if t + 1 < NT_:
                    loads[t + 1] = sp_load(t + 1)
                if 0 <= t - 1 < NT_:
                    l1_states[t - 1] = layer1(t - 1, sp_states.pop(t - 1))
                if t - 2 >= 0:
                    layer23(l1_states.pop(t - 2))
                if t < NT_:
                    sp_states[t] = sp_act(t, loads.pop(t))
                    sp_finish(sp_states[t])

    nc.compile()
    _cache["nc"] = nc
    return nc


# --------------------------------------------------------------------------
# Host-side routing / layout
# --------------------------------------------------------------------------

def _np16():
    if os.environ.get("KERNEL_DT", "float16") == "bfloat16":
        import ml_dtypes
        return ml_dtypes.bfloat16
    return np.float16


def _fm(w, dtype):
    """[K, ...] -> [128, K//128, ...] with contraction index f = o*128 + p."""
    ko = w.shape[0] // 128
    perm = (1, 0) + tuple(range(2, w.ndim + 1))
    return np.ascontiguousarray(
        w.reshape(ko, 128, *w.shape[1:]).transpose(*perm), dtype=dtype)


def _prep_weights(i):
    f32 = np.float32
    f16 = _np16()

    def a(x):
        return np.asarray(x, dtype=f32)

    # f16 blob A: [wsi2 | wso1 | wso2 | wso3] along free dim
    wsi2 = _fm(a(i["Wsi2"]), f16).reshape(128, -1)            # 256
    wso1 = _fm(a(i["Wso1"]), f16).reshape(128, -1)            # 1280
    wso2 = _fm(a(i["Wso2"]), f16).reshape(128, -1)            # 512
    wso3 = _fm(a(i["Wso3"]), f16).reshape(128, -1)            # 2
    wa = np.concatenate([wsi2, wso1, wso2, wso3], axis=1)
    assert wa.shape == (128, 2050)

    # f16 blob B, per-branch-contiguous: for each k [wb1_k | wb2_k | wb3_k]
    wb1 = _fm(a(i["Wb1"]).transpose(1, 0, 2), f16)   # [128, 5, 6, 256]
    wb2 = _fm(a(i["Wb2"]).transpose(1, 0, 2), f16)   # [128, 2, 6, 256]
    wb3 = _fm(a(i["Wb3"]).transpose(1, 0, 2), f16)   # [128, 2, 6, 3]
    parts = []
    for k in range(NBRANCH):
        parts.append(wb1[:, :, k, :].reshape(128, -1))   # 1280
        parts.append(wb2[:, :, k, :].reshape(128, -1))   # 512
        parts.append(wb3[:, :, k, :].reshape(128, -1))   # 6
    wb = np.ascontiguousarray(np.concatenate(parts, axis=1))
    assert wb.shape == (128, NBRANCH * WB_K), wb.shape

    # f32 blob: per-partition bias/scale columns
    wc = np.zeros((128, 40), dtype=f32)
    wc[:, 0:2] = _fm(a(i["bsi1"]), f32)
    wc[:, 2] = a(i["bsi2"])
    wc[:, 3:5] = _fm(a(i["bso1"]), f32)
    wc[:, 5:7] = _fm(a(i["bso2"]), f32)
    wc[:, 7:9] = _fm(a(i["Wsi1"]).reshape(256), f32)
    wc[:, 9:21] = _fm(a(i["bb1"]).T, f32).reshape(128, 12)
    wc[:, 21:33] = _fm(a(i["bb2"]).T, f32).reshape(128, 12)
    wc[0, 33] = a(i["bso3"])[0]
    wc[32:35, 34:40] = a(i["bb3"]).T
    return {"wa": wa, "wb": wb, "wc": wc}


def _route(cmd):
    """Assign rows to (core, slot-position); slot k of every core holds only
    branch-k rows. Returns idx [NCORES, BPC], valid [NCORES, BPC], spill."""
    idx = np.zeros((NCORES, BPC), dtype=np.int64)
    valid = np.zeros((NCORES, BPC), dtype=bool)
    spill = []
    for k in range(NBRANCH):
        rows = np.flatnonzero(cmd == k)
        for c, part in enumerate(np.array_split(rows, NCORES)):
            if len(part) > CAP:
                spill.append(part[CAP:])
                part = part[:CAP]
            idx[c, k * CAP:k * CAP + len(part)] = part
            valid[c, k * CAP:k * CAP + len(part)] = True
    spill = np.concatenate(spill) if spill else np.zeros(0, dtype=np.int64)
    return idx, valid, spill


def _np_reference(i, rows):
    """Exact reference math in numpy for a subset of rows (spill fallback)."""
    f32 = np.float32
    E = np.asarray(i["embedding"], f32)[rows]
    S = np.asarray(i["speed"], f32)[rows]
    cmd = np.asarray(i["command"])[rows].astype(np.int64) - 1
    sp = np.maximum(S @ np.asarray(i["Wsi1"], f32) + np.asarray(i["bsi1"], f32), 0)
    sp = sp @ np.asarray(i["Wsi2"], f32) + np.asarray(i["bsi2"], f32)
    emb = np.concatenate([E, sp], axis=1)
    hs = np.maximum(emb @ np.asarray(i["Wso1"], f32) + np.asarray(i["bso1"], f32), 0)
    hs = np.maximum(hs @ np.asarray(i["Wso2"], f32) + np.asarray(i["bso2"], f32), 0)
    spd = hs @ np.asarray(i["Wso3"], f32) + np.asarray(i["bso3"], f32)
    ctrl = np.zeros((len(rows), 3), f32)
    for k in range(NBRANCH):
        m = cmd == k
        if not m.any():
            continue
        h = np.maximum(emb[m] @ np.asarray(i["Wb1"], f32)[k]
                       + np.asarray(i["bb1"], f32)[k], 0)
        h = np.maximum(h @ np.asarray(i["Wb2"], f32)[k]
                       + np.asarray(i["bb2"], f32)[k], 0)
        z = h @ np.asarray(i["Wb3"], f32)[k] + np.asarray(i["bb3"], f32)[k]
        ctrl[m] = 1.0 / (1.0 + np.exp(-z))
    return ctrl, spd.astype(f32)


# --------------------------------------------------------------------------
# Entry point
# --------------------------------------------------------------------------

LAST_RESULTS = None  # BassKernelResults of the most recent device run


def kernel(embedding, speed, command, **weights):
    global LAST_RESULTS
    inputs = dict(weights)
    inputs.update(embedding=embedding, speed=speed, command=command)

    embedding = np.asarray(embedding, dtype=np.float32)
    speed = np.asarray(speed, dtype=np.float32)
    command_np = np.asarray(command)

    if embedding.shape != (B, D_EMB):
        # Unexpected problem size: fall back to exact host computation.
        ctrl, spd = _np_reference(inputs, np.arange(embedding.shape[0]))
        return ctrl, spd

    cmd = command_np.astype(np.int64) - 1
    idx, valid, spill = _route(cmd)

    w = _prep_weights(inputs)
    f16 = _np16()

    in_maps = []
    for c in range(NCORES):
        rows = idx[c]
        emb_t = embedding[rows].T.astype(f16)                  # [512, BPC]
        emb_t = emb_t.reshape(4, 128, BPC).transpose(1, 0, 2)  # [128, 4, BPC]
        spd_b = speed[rows, 0].astype(f16)                     # [BPC]
        xt = np.empty(XT_ELEMS, dtype=f16)
        for (k, off, nt, eoff) in TILES:
            c0 = k * CAP + off
            blk = xt[eoff:eoff + 128 * 5 * nt].reshape(128, 5, nt)
            blk[:, 0:4, :] = emb_t[:, :, c0:c0 + nt]
            blk[:, 4, :] = spd_b[c0:c0 + nt][None, :]
        m = {"xt": xt}
        m.update(w)
        in_maps.append(m)

    from concourse.bass_utils import run_bass_kernel_spmd

    nc = _build_nc()
    res = run_bass_kernel_spmd(
        nc, in_maps, core_ids=list(range(NCORES)),
        trace=bool(int(os.environ.get("KERNEL_TRACE", "0"))),
    )
    LAST_RESULTS = res

    control = np.zeros((B, 3), dtype=np.float32)
    speed_pred = np.zeros((B, 1), dtype=np.float32)
    for c in range(NCORES):
        o4 = np.asarray(res.results[c]["out4"])
        v = valid[c]
        rows = idx[c][v]
        control[rows] = o4[0:3, v].T
        speed_pred[rows, 0] = o4[3, v]

    if len(spill):
        ctrl_sp, spd_sp = _np_reference(inputs, spill)
        control[spill] = ctrl_sp
        speed_pred[spill] = spd_sp

    return control, speed_pred


# revision 20
# speedup vs baseline: 1.3589x; 1.1026x over previous
"""Trainium2 Bass kernel for nn_CILRSModel (moe_routing).

Strategy:
  - Host-side MoE routing: rows are bucketed by `command` (6 branches) and
    distributed evenly over 8 cores. Each core gets a fixed [6 x CAP] row
    layout so the SPMD kernel statically knows which branch weights apply
    to which batch tile (no on-device routing control flow).
  - Host-side transpose: the embedding is shipped feature-major (partition =
    feature), per batch-tile contiguous, so every matmul operand already has
    the contraction dim on SBUF partitions and every x load is one fast DMA.
    The speed scalar is shipped broadcast across all 128 partitions so the
    speed-MLP's first (K=1) layer runs on the Scalar engine.
  - On device everything is feature-major fp16 (PSUM accumulates fp32).
    Two-stage software pipeline: tile t+1's first-layer matmuls are emitted
    before tile t's second/third layers so eviction latency hides behind
    independent PE work.
  - The two tiny-M output matmuls (control M=3, speed M=1) are packed into
    distinct PE column groups via tile_position so they run concurrently.
  - Outputs return feature-major as out4 = [4, rows] and are scattered back
    to the original row order on host.
"""

import os
import sys

import numpy as np

_TRN_REPO = "/opt/trn_rl_repo"
if _TRN_REPO not in sys.path:
    sys.path.insert(0, _TRN_REPO)

# Problem constants (hardcoded per harness contract)
B = 65536
D_EMB = 512
D_LAT = 128
H = 256
NBRANCH = 6
D_IN = D_EMB + D_LAT  # 640
NCORES = 8
CAP = 1408            # per-core per-branch row capacity (actual max ~1389)
BPC = NBRANCH * CAP   # 8448 rows per core
SLOT_TILES = [(0, 512), (512, 512), (1024, 384)]
WB_K = 1798           # per-branch weight-blob width: 5*256 + 2*256 + 2*3

# tile table: (branch k, col offset, width, xt element offset)
TILES = []
_eoff = 0
for _k in range(NBRANCH):
    for _off, _nt in SLOT_TILES:
        TILES.append((_k, _off, _nt, _eoff))
        _eoff += 128 * 5 * _nt
XT_ELEMS = _eoff

_cache = {}


# --------------------------------------------------------------------------
# Device kernel
# --------------------------------------------------------------------------

def _build_nc():
    if "nc" in _cache:
        return _cache["nc"]

    import concourse.mybir as mybir
    import concourse.tile as tile
    from concourse import bacc

    f32 = mybir.dt.float32
    f16 = getattr(mybir.dt, os.environ.get("KERNEL_DT", "float16"))
    AF = mybir.ActivationFunctionType
    ALU = mybir.AluOpType

    nc = bacc.Bacc("TRN2", target_bir_lowering=False, debug=False,
                   num_devices=NCORES)

    xt = nc.dram_tensor("xt", [XT_ELEMS], f16, kind="ExternalInput")[:]
    wa = nc.dram_tensor("wa", [128, 2050], f16, kind="ExternalInput")[:]
    wb = nc.dram_tensor("wb", [128, NBRANCH * WB_K], f16, kind="ExternalInput")[:]
    wc = nc.dram_tensor("wc", [128, 40], f32, kind="ExternalInput")[:]
    out4 = nc.dram_tensor("out4", [4, BPC], f32, kind="ExternalOutput")[:]

    with tile.TileContext(nc) as tc:
        with (
            tc.tile_pool(name="wpool", bufs=1) as wpool,
            tc.tile_pool(name="xpool", bufs=4) as xpool,
            tc.tile_pool(name="hpool", bufs=2) as hpool,
            tc.tile_pool(name="opool", bufs=1) as opool,
            tc.tile_pool(name="pps", bufs=1, space="PSUM") as pps,
        ):
            wc_s = wpool.tile([128, 40], f32, tag="wc")
            nc.sync.dma_start(out=wc_s[:], in_=wc)
            wa_s = wpool.tile([128, 2050], f16, tag="wa")
            nc.sync.dma_start(out=wa_s[:], in_=wa)
            wb_s = wpool.tile([128, NBRANCH * WB_K], f16, tag="wb")
            # branch 0 weights up front; later branches prefetched in-loop
            nc.sync.dma_start(out=wb_s[:, 0:WB_K], in_=wb[:, 0:WB_K])

            # f16 blob A slices
            def wsi2_sl(o):
                return wa_s[:, o * 128:(o + 1) * 128]

            def wso1_sl(o, j):
                c = 256 + o * 256 + j * 128
                return wa_s[:, c:c + 128]

            def wso2_sl(o, j):
                c = 1536 + o * 256 + j * 128
                return wa_s[:, c:c + 128]

            def wso3_sl(o):
                return wa_s[:, 2048 + o:2049 + o]

            # f16 blob B slices (per-branch-contiguous)
            def wb1_sl(o, k, j):
                c = k * WB_K + o * 256 + j * 128
                return wb_s[:, c:c + 128]

            def wb2_sl(o, k, j):
                c = k * WB_K + 1280 + o * 256 + j * 128
                return wb_s[:, c:c + 128]

            def wb3_sl(o, k):
                c = k * WB_K + 1792 + o * 3
                return wb_s[:, c:c + 3]

            # f32 blob slices (per-partition bias/scale columns)
            def bsi1_c(j):
                return wc_s[:, j:j + 1]

            bsi2_c = wc_s[:, 2:3]

            def bso1_c(j):
                return wc_s[:, 3 + j:4 + j]

            def bso2_c(j):
                return wc_s[:, 5 + j:6 + j]

            def wsi1_c(j):
                return wc_s[:, 7 + j:8 + j]

            def bb1_c(j, k):
                c = 9 + j * 6 + k
                return wc_s[:, c:c + 1]

            def bb2_c(j, k):
                c = 21 + j * 6 + k
                return wc_s[:, c:c + 1]

            bso3_c = wc_s[0:1, 33:34]
            bb3_c = wc_s[32:35, 34:40]   # bb3 lives at partitions 32:35

            ctl_s = opool.tile([35, BPC], f32, tag="octl")  # rows 32:35 used
            spd_s = opool.tile([1, BPC], f32, tag="ospd")

            # warm-up: pull the ACT table load forward (gated only on the tiny
            # wc blob) and keep the PE busy/HAM-warm before real tiles arrive
            warm = wpool.tile([1, 8], f32, tag="warm")
            nc.scalar.activation(warm[:], wc_s[0:1, 0:8], AF.Relu,
                                 bias=bsi1_c(0)[0:1])
            pwarm = pps.tile([128, 2, 512], f32, tag="pp2")
            for _ in range(8):
                nc.tensor.matmul(pwarm[:, 0, :], wa_s[:, 0:128], wa_s[:, 512:1024],
                                 start=True, stop=True)

            def evict_relu(dst, src, bias_ap, on_act):
                if on_act:
                    nc.scalar.activation(dst, src, AF.Relu, bias=bias_ap)
                else:
                    nc.vector.tensor_scalar(dst, src, bias_ap, 0.0, ALU.add, ALU.max)

            def sp_load(t):
                """x tile DMA + branch-weight prefetch."""
                k, off, nt, eoff = TILES[t]
                x_s = xpool.tile([128, 5, nt], f16, tag=f"x{nt}")
                src = xt[eoff:eoff + 128 * 5 * nt].rearrange(
                    "(p o b) -> p o b", p=128, o=5)
                nc.sync.dma_start(out=x_s[:], in_=src)
                if off == 0 and k + 1 < NBRANCH:
                    c = (k + 1) * WB_K
                    nc.sync.dma_start(out=wb_s[:, c:c + WB_K],
                                      in_=wb[:, c:c + WB_K])
                return (x_s, nt)

            def sp_act(t, ld):
                """speed-MLP layer 1 on ScalarE (scale+bias+relu)."""
                x_s, nt = ld
                hsp = hpool.tile([128, 2, nt], f16, tag=f"hsp{nt}")
                for j in range(2):
                    nc.scalar.activation(hsp[:, j, :], x_s[:, 4, :], AF.Relu,
                                         bias=bsi1_c(j), scale=wsi1_c(j))
                return (x_s, hsp, nt)

            def sp_mm(st, q1_tile):
                """speed-MLP layer 2 -> latent overwrites x_s[:,4,:].
                PSUM target aliases the (already-evicted) j1 bank of the
                current cycle's q1 tile - no extra bank allocation."""
                x_s, hsp, nt = st
                p_sp = q1_tile[:, 1, :]
                nc.tensor.matmul(p_sp[:, :nt], wsi2_sl(0), hsp[:, 0, :],
                                 start=True, stop=False)
                nc.tensor.matmul(p_sp[:, :nt], wsi2_sl(1), hsp[:, 1, :],
                                 start=False, stop=True)
                nc.vector.tensor_scalar(x_s[:, 4, :], p_sp[:, :nt], bsi2_c,
                                        None, ALU.add)

            def layer1(t, st):
                """Layer-1 of both heads + h1/g1 evictions."""
                x_s, hsp, nt = st
                k, off, _, _ = TILES[t]

                p1 = pps.tile([128, 2, 512], f32, tag="pp1")
                for j in range(2):
                    for o in range(4):
                        nc.tensor.matmul(p1[:, j, :nt], wso1_sl(o, j),
                                         x_s[:, o, :],
                                         start=(o == 0), stop=False)
                q1 = pps.tile([128, 2, 512], f32, tag="pq1")
                for j in range(2):
                    for o in range(4):
                        nc.tensor.matmul(q1[:, j, :nt], wb1_sl(o, k, j),
                                         x_s[:, o, :],
                                         start=(o == 0), stop=False)
                # latent subtile finishers (x_s[:,4,:] written last cycle)
                for j in range(2):
                    nc.tensor.matmul(p1[:, j, :nt], wso1_sl(4, j),
                                     x_s[:, 4, :], start=False, stop=True)
                for j in range(2):
                    nc.tensor.matmul(q1[:, j, :nt], wb1_sl(4, k, j),
                                     x_s[:, 4, :], start=False, stop=True)

                h1 = hpool.tile([128, 2, nt], f16, tag=f"h1{nt}")
                evict_relu(h1[:, 0, :], p1[:, 0, :nt], bso1_c(0), True)
                evict_relu(h1[:, 1, :], p1[:, 1, :nt], bso1_c(1), False)
                g1 = hpool.tile([128, 2, nt], f16, tag=f"g1{nt}")
                evict_relu(g1[:, 0, :], q1[:, 0, :nt], bb1_c(0, k), True)
                evict_relu(g1[:, 1, :], q1[:, 1, :nt], bb1_c(1, k), False)
                return (k, off, nt, h1, g1, q1)

            def l23_mm(st):
                """Layer-2 matmuls + h2/g2 evictions."""
                k, off, nt, h1, g1, _ = st
                p2 = pps.tile([128, 2, 512], f32, tag="pp2")
                for j in range(2):
                    for o in range(2):
                        nc.tensor.matmul(p2[:, j, :nt], wso2_sl(o, j),
                                         h1[:, o, :],
                                         start=(o == 0), stop=(o == 1))
                q2 = pps.tile([128, 2, 512], f32, tag="pq2")
                for j in range(2):
                    for o in range(2):
                        nc.tensor.matmul(q2[:, j, :nt], wb2_sl(o, k, j),
                                         g1[:, o, :],
                                         start=(o == 0), stop=(o == 1))

                h2 = hpool.tile([128, 2, nt], f16, tag=f"h2{nt}")
                evict_relu(h2[:, 0, :], p2[:, 0, :nt], bso2_c(0), True)
                evict_relu(h2[:, 1, :], p2[:, 1, :nt], bso2_c(1), False)
                g2 = hpool.tile([128, 2, nt], f16, tag=f"g2{nt}")
                evict_relu(g2[:, 0, :], q2[:, 0, :nt], bb2_c(0, k), False)
                evict_relu(g2[:, 1, :], q2[:, 1, :nt], bb2_c(1, k), False)
                return (k, off, nt, h2, g2, q2)

            def l23_tail(st):
                """Output layer into q2's (evicted) j0 bank + final evicts."""
                k, off, nt, h2, g2, q2 = st
                c0 = k * CAP + off
                cols = slice(c0, c0 + nt)

                # spd (M=1) in col group 0, ctrl (M=3) in col group 1
                nc.tensor.matmul(q2[0:1, 0, :nt], wso3_sl(0), h2[:, 0, :],
                                 start=True, stop=False, tile_position=(0, 0))
                nc.tensor.matmul(q2[0:1, 0, :nt], wso3_sl(1), h2[:, 1, :],
                                 start=False, stop=True, tile_position=(0, 0))
                nc.tensor.matmul(q2[32:35, 0, :nt], wb3_sl(0, k), g2[:, 0, :],
                                 start=True, stop=False, tile_position=(0, 32))
                nc.tensor.matmul(q2[32:35, 0, :nt], wb3_sl(1, k), g2[:, 1, :],
                                 start=False, stop=True, tile_position=(0, 32))
                nc.vector.tensor_scalar(spd_s[:, cols], q2[0:1, 0, :nt],
                                        bso3_c, None, ALU.add)
                nc.scalar.activation(ctl_s[32:35, cols], q2[32:35, 0, :nt],
                                     AF.Sigmoid, bias=bb3_c[:, k:k + 1])
                # stream results out once a branch slot's last tile is done
                if off == 1024:
                    kcols = slice(k * CAP, (k + 1) * CAP)
                    nc.sync.dma_start(out=out4[0:3, kcols],
                                      in_=ctl_s[32:35, kcols])
                    nc.sync.dma_start(out=out4[3:4, kcols],
                                      in_=spd_s[:, kcols])

            # pipeline: load(t+1) | l23_mm(t-2) | layer1(t-1) | tail(t-2)
            #           | sp_act(t+1) | sp_mm(t)
            NT_ = len(TILES)
            loads = {0: sp_load(0)}
            sp_states = {0: sp_act(0, loads.pop(0))}
            l1_states = {}
            l23_states = {}
            for t in range(NT_ + 2):
                if t + 1 < NT_:
                    loads[t + 1] = sp_load(t + 1)
                if t - 2 >= 0:
                    l23_states[t - 2] = l23_mm(l1_states.pop(t - 2))
                if 0 <= t - 1 < NT_:
                    l1_states[t - 1] = layer1(t - 1, sp_states.pop(t - 1))
                if t - 2 >= 0:
                    l23_tail(l23_states.pop(t - 2))
                if 0 < t + 1 < NT_:
                    sp_states[t + 1] = sp_act(t + 1, loads.pop(t + 1))
                if t < NT_:
                    if t >= 1:
                        q1_t = l1_states[t - 1][5]
                    else:
                        q1_t = pps.tile([128, 2, 512], f32, tag="pq1")
                    sp_mm(sp_states[t], q1_t)

    nc.compile()
    _cache["nc"] = nc
    return nc


# --------------------------------------------------------------------------
# Host-side routing / layout
# --------------------------------------------------------------------------

def _np16():
    if os.environ.get("KERNEL_DT", "float16") == "bfloat16":
        import ml_dtypes
        return ml_dtypes.bfloat16
    return np.float16


def _fm(w, dtype):
    """[K, ...] -> [128, K//128, ...] with contraction index f = o*128 + p."""
    ko = w.shape[0] // 128
    perm = (1, 0) + tuple(range(2, w.ndim + 1))
    return np.ascontiguousarray(
        w.reshape(ko, 128, *w.shape[1:]).transpose(*perm), dtype=dtype)


def _prep_weights(i):
    f32 = np.float32
    f16 = _np16()

    def a(x):
        return np.asarray(x, dtype=f32)

    # f16 blob A: [wsi2 | wso1 | wso2 | wso3] along free dim
    wsi2 = _fm(a(i["Wsi2"]), f16).reshape(128, -1)            # 256
    wso1 = _fm(a(i["Wso1"]), f16).reshape(128, -1)            # 1280
    wso2 = _fm(a(i["Wso2"]), f16).reshape(128, -1)            # 512
    wso3 = _fm(a(i["Wso3"]), f16).reshape(128, -1)            # 2
    wa = np.concatenate([wsi2, wso1, wso2, wso3], axis=1)
    assert wa.shape == (128, 2050)

    # f16 blob B, per-branch-contiguous: for each k [wb1_k | wb2_k | wb3_k]
    wb1 = _fm(a(i["Wb1"]).transpose(1, 0, 2), f16)   # [128, 5, 6, 256]
    wb2 = _fm(a(i["Wb2"]).transpose(1, 0, 2), f16)   # [128, 2, 6, 256]
    wb3 = _fm(a(i["Wb3"]).transpose(1, 0, 2), f16)   # [128, 2, 6, 3]
    parts = []
    for k in range(NBRANCH):
        parts.append(wb1[:, :, k, :].reshape(128, -1))   # 1280
        parts.append(wb2[:, :, k, :].reshape(128, -1))   # 512
        parts.append(wb3[:, :, k, :].reshape(128, -1))   # 6
    wb = np.ascontiguousarray(np.concatenate(parts, axis=1))
    assert wb.shape == (128, NBRANCH * WB_K), wb.shape

    # f32 blob: per-partition bias/scale columns
    wc = np.zeros((128, 40), dtype=f32)
    wc[:, 0:2] = _fm(a(i["bsi1"]), f32)
    wc[:, 2] = a(i["bsi2"])
    wc[:, 3:5] = _fm(a(i["bso1"]), f32)
    wc[:, 5:7] = _fm(a(i["bso2"]), f32)
    wc[:, 7:9] = _fm(a(i["Wsi1"]).reshape(256), f32)
    wc[:, 9:21] = _fm(a(i["bb1"]).T, f32).reshape(128, 12)
    wc[:, 21:33] = _fm(a(i["bb2"]).T, f32).reshape(128, 12)
    wc[0, 33] = a(i["bso3"])[0]
    wc[32:35, 34:40] = a(i["bb3"]).T
    return {"wa": wa, "wb": wb, "wc": wc}


def _route(cmd):
    """Assign rows to (core, slot-position); slot k of every core holds only
    branch-k rows. Returns idx [NCORES, BPC], valid [NCORES, BPC], spill."""
    idx = np.zeros((NCORES, BPC), dtype=np.int64)
    valid = np.zeros((NCORES, BPC), dtype=bool)
    spill = []
    for k in range(NBRANCH):
        rows = np.flatnonzero(cmd == k)
        for c, part in enumerate(np.array_split(rows, NCORES)):
            if len(part) > CAP:
                spill.append(part[CAP:])
                part = part[:CAP]
            idx[c, k * CAP:k * CAP + len(part)] = part
            valid[c, k * CAP:k * CAP + len(part)] = True
    spill = np.concatenate(spill) if spill else np.zeros(0, dtype=np.int64)
    return idx, valid, spill


def _np_reference(i, rows):
    """Exact reference math in numpy for a subset of rows (spill fallback)."""
    f32 = np.float32
    E = np.asarray(i["embedding"], f32)[rows]
    S = np.asarray(i["speed"], f32)[rows]
    cmd = np.asarray(i["command"])[rows].astype(np.int64) - 1
    sp = np.maximum(S @ np.asarray(i["Wsi1"], f32) + np.asarray(i["bsi1"], f32), 0)
    sp = sp @ np.asarray(i["Wsi2"], f32) + np.asarray(i["bsi2"], f32)
    emb = np.concatenate([E, sp], axis=1)
    hs = np.maximum(emb @ np.asarray(i["Wso1"], f32) + np.asarray(i["bso1"], f32), 0)
    hs = np.maximum(hs @ np.asarray(i["Wso2"], f32) + np.asarray(i["bso2"], f32), 0)
    spd = hs @ np.asarray(i["Wso3"], f32) + np.asarray(i["bso3"], f32)
    ctrl = np.zeros((len(rows), 3), f32)
    for k in range(NBRANCH):
        m = cmd == k
        if not m.any():
            continue
        h = np.maximum(emb[m] @ np.asarray(i["Wb1"], f32)[k]
                       + np.asarray(i["bb1"], f32)[k], 0)
        h = np.maximum(h @ np.asarray(i["Wb2"], f32)[k]
                       + np.asarray(i["bb2"], f32)[k], 0)
        z = h @ np.asarray(i["Wb3"], f32)[k] + np.asarray(i["bb3"], f32)[k]
        ctrl[m] = 1.0 / (1.0 + np.exp(-z))
    return ctrl, spd.astype(f32)


# --------------------------------------------------------------------------
# Entry point
# --------------------------------------------------------------------------

LAST_RESULTS = None  # BassKernelResults of the most recent device run


def kernel(embedding, speed, command, **weights):
    global LAST_RESULTS
    inputs = dict(weights)
    inputs.update(embedding=embedding, speed=speed, command=command)

    embedding = np.asarray(embedding, dtype=np.float32)
    speed = np.asarray(speed, dtype=np.float32)
    command_np = np.asarray(command)

    if embedding.shape != (B, D_EMB):
        # Unexpected problem size: fall back to exact host computation.
        ctrl, spd = _np_reference(inputs, np.arange(embedding.shape[0]))
        return ctrl, spd

    cmd = command_np.astype(np.int64) - 1
    idx, valid, spill = _route(cmd)

    w = _prep_weights(inputs)
    f16 = _np16()

    in_maps = []
    for c in range(NCORES):
        rows = idx[c]
        emb_t = embedding[rows].T.astype(f16)                  # [512, BPC]
        emb_t = emb_t.reshape(4, 128, BPC).transpose(1, 0, 2)  # [128, 4, BPC]
        spd_b = speed[rows, 0].astype(f16)                     # [BPC]
        xt = np.empty(XT_ELEMS, dtype=f16)
        for (k, off, nt, eoff) in TILES:
            c0 = k * CAP + off
            blk = xt[eoff:eoff + 128 * 5 * nt].reshape(128, 5, nt)
            blk[:, 0:4, :] = emb_t[:, :, c0:c0 + nt]
            blk[:, 4, :] = spd_b[c0:c0 + nt][None, :]
        m = {"xt": xt}
        m.update(w)
        in_maps.append(m)

    from concourse.bass_utils import run_bass_kernel_spmd

    nc = _build_nc()
    res = run_bass_kernel_spmd(
        nc, in_maps, core_ids=list(range(NCORES)),
        trace=bool(int(os.environ.get("KERNEL_TRACE", "0"))),
    )
    LAST_RESULTS = res

    control = np.zeros((B, 3), dtype=np.float32)
    speed_pred = np.zeros((B, 1), dtype=np.float32)
    for c in range(NCORES):
        o4 = np.asarray(res.results[c]["out4"])
        v = valid[c]
        rows = idx[c][v]
        control[rows] = o4[0:3, v].T
        speed_pred[rows, 0] = o4[3, v]

    if len(spill):
        ctrl_sp, spd_sp = _np_reference(inputs, spill)
        control[spill] = ctrl_sp
        speed_pred[spill] = spd_sp

    return control, speed_pred


# revision 21
# speedup vs baseline: 1.3630x; 1.0030x over previous
"""Trainium2 Bass kernel for nn_CILRSModel (moe_routing).

Strategy:
  - Host-side MoE routing: rows are bucketed by `command` (6 branches) and
    distributed evenly over 8 cores. Each core gets a fixed [6 x CAP] row
    layout so the SPMD kernel statically knows which branch weights apply
    to which batch tile (no on-device routing control flow).
  - Host-side transpose: the embedding is shipped feature-major (partition =
    feature), per batch-tile contiguous, so every matmul operand already has
    the contraction dim on SBUF partitions and every x load is one fast DMA.
    The speed scalar is shipped broadcast across all 128 partitions so the
    speed-MLP's first (K=1) layer runs on the Scalar engine.
  - On device everything is feature-major fp16 (PSUM accumulates fp32).
    Two-stage software pipeline: tile t+1's first-layer matmuls are emitted
    before tile t's second/third layers so eviction latency hides behind
    independent PE work.
  - The two tiny-M output matmuls (control M=3, speed M=1) are packed into
    distinct PE column groups via tile_position so they run concurrently.
  - Outputs return feature-major as out4 = [4, rows] and are scattered back
    to the original row order on host.
"""

import os
import sys

import numpy as np

_TRN_REPO = "/opt/trn_rl_repo"
if _TRN_REPO not in sys.path:
    sys.path.insert(0, _TRN_REPO)

# Problem constants (hardcoded per harness contract)
B = 65536
D_EMB = 512
D_LAT = 128
H = 256
NBRANCH = 6
D_IN = D_EMB + D_LAT  # 640
NCORES = 8
CAP = 1408            # per-core per-branch row capacity (actual max ~1389)
BPC = NBRANCH * CAP   # 8448 rows per core
SLOT_TILES = [(0, 512), (512, 512), (1024, 384)]
WB_K = 1798           # per-branch weight-blob width: 5*256 + 2*256 + 2*3

# tile table: (branch k, col offset, width, xt element offset)
TILES = []
_eoff = 0
for _k in range(NBRANCH):
    for _off, _nt in SLOT_TILES:
        TILES.append((_k, _off, _nt, _eoff))
        _eoff += 128 * 5 * _nt
XT_ELEMS = _eoff

_cache = {}


# --------------------------------------------------------------------------
# Device kernel
# --------------------------------------------------------------------------

def _build_nc():
    if "nc" in _cache:
        return _cache["nc"]

    import concourse.mybir as mybir
    import concourse.tile as tile
    from concourse import bacc

    f32 = mybir.dt.float32
    f16 = getattr(mybir.dt, os.environ.get("KERNEL_DT", "float16"))
    AF = mybir.ActivationFunctionType
    ALU = mybir.AluOpType

    nc = bacc.Bacc("TRN2", target_bir_lowering=False, debug=False,
                   num_devices=NCORES)

    xt = nc.dram_tensor("xt", [XT_ELEMS], f16, kind="ExternalInput")[:]
    wa = nc.dram_tensor("wa", [128, 2050], f16, kind="ExternalInput")[:]
    wb = nc.dram_tensor("wb", [128, NBRANCH * WB_K], f16, kind="ExternalInput")[:]
    wc = nc.dram_tensor("wc", [128, 40], f32, kind="ExternalInput")[:]
    out4 = nc.dram_tensor("out4", [4, BPC], f32, kind="ExternalOutput")[:]

    with tile.TileContext(nc) as tc:
        with (
            tc.tile_pool(name="wpool", bufs=1) as wpool,
            tc.tile_pool(name="xpool", bufs=4) as xpool,
            tc.tile_pool(name="hpool", bufs=2) as hpool,
            tc.tile_pool(name="opool", bufs=1) as opool,
            tc.tile_pool(name="pps", bufs=1, space="PSUM") as pps,
        ):
            wc_s = wpool.tile([128, 40], f32, tag="wc")
            nc.sync.dma_start(out=wc_s[:], in_=wc)
            wa_s = wpool.tile([128, 2050], f16, tag="wa")
            nc.sync.dma_start(out=wa_s[:], in_=wa)
            wb_s = wpool.tile([128, NBRANCH * WB_K], f16, tag="wb")
            # branch 0 weights up front; later branches prefetched in-loop
            nc.sync.dma_start(out=wb_s[:, 0:WB_K], in_=wb[:, 0:WB_K])

            # f16 blob A slices
            def wsi2_sl(o):
                return wa_s[:, o * 128:(o + 1) * 128]

            def wso1_sl(o, j):
                c = 256 + o * 256 + j * 128
                return wa_s[:, c:c + 128]

            def wso2_sl(o, j):
                c = 1536 + o * 256 + j * 128
                return wa_s[:, c:c + 128]

            def wso3_sl(o):
                return wa_s[:, 2048 + o:2049 + o]

            # f16 blob B slices (per-branch-contiguous)
            def wb1_sl(o, k, j):
                c = k * WB_K + o * 256 + j * 128
                return wb_s[:, c:c + 128]

            def wb2_sl(o, k, j):
                c = k * WB_K + 1280 + o * 256 + j * 128
                return wb_s[:, c:c + 128]

            def wb3_sl(o, k):
                c = k * WB_K + 1792 + o * 3
                return wb_s[:, c:c + 3]

            # f32 blob slices (per-partition bias/scale columns)
            def bsi1_c(j):
                return wc_s[:, j:j + 1]

            bsi2_c = wc_s[:, 2:3]

            def bso1_c(j):
                return wc_s[:, 3 + j:4 + j]

            def bso2_c(j):
                return wc_s[:, 5 + j:6 + j]

            def wsi1_c(j):
                return wc_s[:, 7 + j:8 + j]

            def bb1_c(j, k):
                c = 9 + j * 6 + k
                return wc_s[:, c:c + 1]

            def bb2_c(j, k):
                c = 21 + j * 6 + k
                return wc_s[:, c:c + 1]

            bso3_c = wc_s[0:1, 33:34]
            bb3_c = wc_s[32:35, 34:40]   # bb3 lives at partitions 32:35

            ctl_s = opool.tile([35, BPC], f32, tag="octl")  # rows 32:35 used
            spd_s = opool.tile([1, BPC], f32, tag="ospd")

            # warm-up: pull the ACT table load forward (gated only on the tiny
            # wc blob) and keep the PE busy/HAM-warm before real tiles arrive
            warm = wpool.tile([1, 8], f32, tag="warm")
            nc.scalar.activation(warm[:], wc_s[0:1, 0:8], AF.Relu,
                                 bias=bsi1_c(0)[0:1])
            pwarm = pps.tile([128, 2, 512], f32, tag="pp2")
            for _ in range(8):
                nc.tensor.matmul(pwarm[:, 0, :], wa_s[:, 0:128], wa_s[:, 512:1024],
                                 start=True, stop=True)

            def evict_relu(dst, src, bias_ap, on_act):
                if on_act:
                    nc.scalar.activation(dst, src, AF.Relu, bias=bias_ap)
                else:
                    nc.vector.tensor_scalar(dst, src, bias_ap, 0.0, ALU.add, ALU.max)

            def sp_load(t):
                """x tile DMA + branch-weight prefetch."""
                k, off, nt, eoff = TILES[t]
                x_s = xpool.tile([128, 5, nt], f16, tag=f"x{nt}")
                src = xt[eoff:eoff + 128 * 5 * nt].rearrange(
                    "(p o b) -> p o b", p=128, o=5)
                nc.sync.dma_start(out=x_s[:], in_=src)
                if off == 0 and k + 1 < NBRANCH:
                    c = (k + 1) * WB_K
                    nc.sync.dma_start(out=wb_s[:, c:c + WB_K],
                                      in_=wb[:, c:c + WB_K])
                return (x_s, nt)

            def sp_act(t, ld):
                """speed-MLP layer 1 on ScalarE (scale+bias+relu)."""
                x_s, nt = ld
                hsp = hpool.tile([128, 2, nt], f16, tag=f"hsp{nt}")
                for j in range(2):
                    nc.scalar.activation(hsp[:, j, :], x_s[:, 4, :], AF.Relu,
                                         bias=bsi1_c(j), scale=wsi1_c(j))
                return (x_s, hsp, nt)

            def sp_mm(st, q1_tile):
                """speed-MLP layer 2 -> latent overwrites x_s[:,4,:].
                PSUM target aliases the (already-evicted) j1 bank of the
                current cycle's q1 tile - no extra bank allocation."""
                x_s, hsp, nt = st
                p_sp = q1_tile[:, 1, :]
                nc.tensor.matmul(p_sp[:, :nt], wsi2_sl(0), hsp[:, 0, :],
                                 start=True, stop=False)
                nc.tensor.matmul(p_sp[:, :nt], wsi2_sl(1), hsp[:, 1, :],
                                 start=False, stop=True)
                nc.vector.tensor_scalar(x_s[:, 4, :], p_sp[:, :nt], bsi2_c,
                                        None, ALU.add)

            def layer1(t, st):
                """Layer-1 of both heads + h1/g1 evictions."""
                x_s, hsp, nt = st
                k, off, _, _ = TILES[t]

                p1 = pps.tile([128, 2, 512], f32, tag="pp1")
                for j in range(2):
                    for o in range(4):
                        nc.tensor.matmul(p1[:, j, :nt], wso1_sl(o, j),
                                         x_s[:, o, :],
                                         start=(o == 0), stop=False)
                q1 = pps.tile([128, 2, 512], f32, tag="pq1")
                for j in range(2):
                    for o in range(4):
                        nc.tensor.matmul(q1[:, j, :nt], wb1_sl(o, k, j),
                                         x_s[:, o, :],
                                         start=(o == 0), stop=False)
                # latent subtile finishers (x_s[:,4,:] written last cycle)
                for j in range(2):
                    nc.tensor.matmul(p1[:, j, :nt], wso1_sl(4, j),
                                     x_s[:, 4, :], start=False, stop=True)
                for j in range(2):
                    nc.tensor.matmul(q1[:, j, :nt], wb1_sl(4, k, j),
                                     x_s[:, 4, :], start=False, stop=True)

                h1 = hpool.tile([128, 2, nt], f16, tag=f"h1{nt}")
                evict_relu(h1[:, 0, :], p1[:, 0, :nt], bso1_c(0), True)
                evict_relu(h1[:, 1, :], p1[:, 1, :nt], bso1_c(1), False)
                g1 = hpool.tile([128, 2, nt], f16, tag=f"g1{nt}")
                evict_relu(g1[:, 0, :], q1[:, 0, :nt], bb1_c(0, k), True)
                evict_relu(g1[:, 1, :], q1[:, 1, :nt], bb1_c(1, k), False)
                return (k, off, nt, h1, g1, q1)

            def l23_mm(st):
                """Layer-2 matmuls + h2/g2 evictions."""
                k, off, nt, h1, g1, _ = st
                p2 = pps.tile([128, 2, 512], f32, tag="pp2")
                for j in range(2):
                    for o in range(2):
                        nc.tensor.matmul(p2[:, j, :nt], wso2_sl(o, j),
                                         h1[:, o, :],
                                         start=(o == 0), stop=(o == 1))
                q2 = pps.tile([128, 2, 512], f32, tag="pq2")
                for j in range(2):
                    for o in range(2):
                        nc.tensor.matmul(q2[:, j, :nt], wb2_sl(o, k, j),
                                         g1[:, o, :],
                                         start=(o == 0), stop=(o == 1))

                h2 = hpool.tile([128, 2, nt], f16, tag=f"h2{nt}")
                evict_relu(h2[:, 0, :], p2[:, 0, :nt], bso2_c(0), True)
                evict_relu(h2[:, 1, :], p2[:, 1, :nt], bso2_c(1), False)
                g2 = hpool.tile([128, 2, nt], f16, tag=f"g2{nt}")
                evict_relu(g2[:, 0, :], q2[:, 0, :nt], bb2_c(0, k), True)
                evict_relu(g2[:, 1, :], q2[:, 1, :nt], bb2_c(1, k), False)
                return (k, off, nt, h2, g2, q2)

            def l23_tail(st):
                """Output layer into q2's (evicted) j0 bank + final evicts."""
                k, off, nt, h2, g2, q2 = st
                c0 = k * CAP + off
                cols = slice(c0, c0 + nt)

                # spd (M=1) in col group 0, ctrl (M=3) in col group 1
                nc.tensor.matmul(q2[0:1, 0, :nt], wso3_sl(0), h2[:, 0, :],
                                 start=True, stop=False, tile_position=(0, 0))
                nc.tensor.matmul(q2[0:1, 0, :nt], wso3_sl(1), h2[:, 1, :],
                                 start=False, stop=True, tile_position=(0, 0))
                nc.tensor.matmul(q2[32:35, 0, :nt], wb3_sl(0, k), g2[:, 0, :],
                                 start=True, stop=False, tile_position=(0, 32))
                nc.tensor.matmul(q2[32:35, 0, :nt], wb3_sl(1, k), g2[:, 1, :],
                                 start=False, stop=True, tile_position=(0, 32))
                nc.vector.tensor_scalar(spd_s[:, cols], q2[0:1, 0, :nt],
                                        bso3_c, None, ALU.add)
                nc.scalar.activation(ctl_s[32:35, cols], q2[32:35, 0, :nt],
                                     AF.Sigmoid, bias=bb3_c[:, k:k + 1])
                # stream results out once a branch slot's last tile is done
                if off == 1024:
                    kcols = slice(k * CAP, (k + 1) * CAP)
                    nc.sync.dma_start(out=out4[0:3, kcols],
                                      in_=ctl_s[32:35, kcols])
                    nc.sync.dma_start(out=out4[3:4, kcols],
                                      in_=spd_s[:, kcols])

            # pipeline: load(t+1) | l23_mm(t-2) | layer1(t-1) | tail(t-2)
            #           | sp_act(t+1) | sp_mm(t)
            NT_ = len(TILES)
            loads = {0: sp_load(0)}
            sp_states = {0: sp_act(0, loads.pop(0))}
            l1_states = {}
            l23_states = {}
            for t in range(NT_ + 2):
                if t + 1 < NT_:
                    loads[t + 1] = sp_load(t + 1)
                if t - 2 >= 0:
                    l23_states[t - 2] = l23_mm(l1_states.pop(t - 2))
                if 0 <= t - 1 < NT_:
                    l1_states[t - 1] = layer1(t - 1, sp_states.pop(t - 1))
                if t - 2 >= 0:
                    l23_tail(l23_states.pop(t - 2))
                if 0 < t + 1 < NT_:
                    sp_states[t + 1] = sp_act(t + 1, loads.pop(t + 1))
                if t < NT_:
                    if t >= 1:
                        q1_t = l1_states[t - 1][5]
                    else:
                        q1_t = pps.tile([128, 2, 512], f32, tag="pq1")
                    sp_mm(sp_states[t], q1_t)

    nc.compile()
    _cache["nc"] = nc
    return nc


# --------------------------------------------------------------------------
# Host-side routing / layout
# --------------------------------------------------------------------------

def _np16():
    if os.environ.get("KERNEL_DT", "float16") == "bfloat16":
        import ml_dtypes
        return ml_dtypes.bfloat16
    return np.float16


def _fm(w, dtype):
    """[K, ...] -> [128, K//128, ...] with contraction index f = o*128 + p."""
    ko = w.shape[0] // 128
    perm = (1, 0) + tuple(range(2, w.ndim + 1))
    return np.ascontiguousarray(
        w.reshape(ko, 128, *w.shape[1:]).transpose(*perm), dtype=dtype)


def _prep_weights(i):
    f32 = np.float32
    f16 = _np16()

    def a(x):
        return np.asarray(x, dtype=f32)

    # f16 blob A: [wsi2 | wso1 | wso2 | wso3] along free dim
    wsi2 = _fm(a(i["Wsi2"]), f16).reshape(128, -1)            # 256
    wso1 = _fm(a(i["Wso1"]), f16).reshape(128, -1)            # 1280
    wso2 = _fm(a(i["Wso2"]), f16).reshape(128, -1)            # 512
    wso3 = _fm(a(i["Wso3"]), f16).reshape(128, -1)            # 2
    wa = np.concatenate([wsi2, wso1, wso2, wso3], axis=1)
    assert wa.shape == (128, 2050)

    # f16 blob B, per-branch-contiguous: for each k [wb1_k | wb2_k | wb3_k]
    wb1 = _fm(a(i["Wb1"]).transpose(1, 0, 2), f16)   # [128, 5, 6, 256]
    wb2 = _fm(a(i["Wb2"]).transpose(1, 0, 2), f16)   # [128, 2, 6, 256]
    wb3 = _fm(a(i["Wb3"]).transpose(1, 0, 2), f16)   # [128, 2, 6, 3]
    parts = []
    for k in range(NBRANCH):
        parts.append(wb1[:, :, k, :].reshape(128, -1))   # 1280
        parts.append(wb2[:, :, k, :].reshape(128, -1))   # 512
        parts.append(wb3[:, :, k, :].reshape(128, -1))   # 6
    wb = np.ascontiguousarray(np.concatenate(parts, axis=1))
    assert wb.shape == (128, NBRANCH * WB_K), wb.shape

    # f32 blob: per-partition bias/scale columns
    wc = np.zeros((128, 40), dtype=f32)
    wc[:, 0:2] = _fm(a(i["bsi1"]), f32)
    wc[:, 2] = a(i["bsi2"])
    wc[:, 3:5] = _fm(a(i["bso1"]), f32)
    wc[:, 5:7] = _fm(a(i["bso2"]), f32)
    wc[:, 7:9] = _fm(a(i["Wsi1"]).reshape(256), f32)
    wc[:, 9:21] = _fm(a(i["bb1"]).T, f32).reshape(128, 12)
    wc[:, 21:33] = _fm(a(i["bb2"]).T, f32).reshape(128, 12)
    wc[0, 33] = a(i["bso3"])[0]
    wc[32:35, 34:40] = a(i["bb3"]).T
    return {"wa": wa, "wb": wb, "wc": wc}


def _route(cmd):
    """Assign rows to (core, slot-position); slot k of every core holds only
    branch-k rows. Returns idx [NCORES, BPC], valid [NCORES, BPC], spill."""
    idx = np.zeros((NCORES, BPC), dtype=np.int64)
    valid = np.zeros((NCORES, BPC), dtype=bool)
    spill = []
    for k in range(NBRANCH):
        rows = np.flatnonzero(cmd == k)
        for c, part in enumerate(np.array_split(rows, NCORES)):
            if len(part) > CAP:
                spill.append(part[CAP:])
                part = part[:CAP]
            idx[c, k * CAP:k * CAP + len(part)] = part
            valid[c, k * CAP:k * CAP + len(part)] = True
    spill = np.concatenate(spill) if spill else np.zeros(0, dtype=np.int64)
    return idx, valid, spill


def _np_reference(i, rows):
    """Exact reference math in numpy for a subset of rows (spill fallback)."""
    f32 = np.float32
    E = np.asarray(i["embedding"], f32)[rows]
    S = np.asarray(i["speed"], f32)[rows]
    cmd = np.asarray(i["command"])[rows].astype(np.int64) - 1
    sp = np.maximum(S @ np.asarray(i["Wsi1"], f32) + np.asarray(i["bsi1"], f32), 0)
    sp = sp @ np.asarray(i["Wsi2"], f32) + np.asarray(i["bsi2"], f32)
    emb = np.concatenate([E, sp], axis=1)
    hs = np.maximum(emb @ np.asarray(i["Wso1"], f32) + np.asarray(i["bso1"], f32), 0)
    hs = np.maximum(hs @ np.asarray(i["Wso2"], f32) + np.asarray(i["bso2"], f32), 0)
    spd = hs @ np.asarray(i["Wso3"], f32) + np.asarray(i["bso3"], f32)
    ctrl = np.zeros((len(rows), 3), f32)
    for k in range(NBRANCH):
        m = cmd == k
        if not m.any():
            continue
        h = np.maximum(emb[m] @ np.asarray(i["Wb1"], f32)[k]
                       + np.asarray(i["bb1"], f32)[k], 0)
        h = np.maximum(h @ np.asarray(i["Wb2"], f32)[k]
                       + np.asarray(i["bb2"], f32)[k], 0)
        z = h @ np.asarray(i["Wb3"], f32)[k] + np.asarray(i["bb3"], f32)[k]
        ctrl[m] = 1.0 / (1.0 + np.exp(-z))
    return ctrl, spd.astype(f32)


# --------------------------------------------------------------------------
# Entry point
# --------------------------------------------------------------------------

LAST_RESULTS = None  # BassKernelResults of the most recent device run


def kernel(embedding, speed, command, **weights):
    global LAST_RESULTS
    inputs = dict(weights)
    inputs.update(embedding=embedding, speed=speed, command=command)

    embedding = np.asarray(embedding, dtype=np.float32)
    speed = np.asarray(speed, dtype=np.float32)
    command_np = np.asarray(command)

    if embedding.shape != (B, D_EMB):
        # Unexpected problem size: fall back to exact host computation.
        ctrl, spd = _np_reference(inputs, np.arange(embedding.shape[0]))
        return ctrl, spd

    cmd = command_np.astype(np.int64) - 1
    idx, valid, spill = _route(cmd)

    w = _prep_weights(inputs)
    f16 = _np16()

    in_maps = []
    for c in range(NCORES):
        rows = idx[c]
        emb_t = embedding[rows].T.astype(f16)                  # [512, BPC]
        emb_t = emb_t.reshape(4, 128, BPC).transpose(1, 0, 2)  # [128, 4, BPC]
        spd_b = speed[rows, 0].astype(f16)                     # [BPC]
        xt = np.empty(XT_ELEMS, dtype=f16)
        for (k, off, nt, eoff) in TILES:
            c0 = k * CAP + off
            blk = xt[eoff:eoff + 128 * 5 * nt].reshape(128, 5, nt)
            blk[:, 0:4, :] = emb_t[:, :, c0:c0 + nt]
            blk[:, 4, :] = spd_b[c0:c0 + nt][None, :]
        m = {"xt": xt}
        m.update(w)
        in_maps.append(m)

    from concourse.bass_utils import run_bass_kernel_spmd

    nc = _build_nc()
    res = run_bass_kernel_spmd(
        nc, in_maps, core_ids=list(range(NCORES)),
        trace=bool(int(os.environ.get("KERNEL_TRACE", "0"))),
    )
    LAST_RESULTS = res

    control = np.zeros((B, 3), dtype=np.float32)
    speed_pred = np.zeros((B, 1), dtype=np.float32)
    for c in range(NCORES):
        o4 = np.asarray(res.results[c]["out4"])
        v = valid[c]
        rows = idx[c][v]
        control[rows] = o4[0:3, v].T
        speed_pred[rows, 0] = o4[3, v]

    if len(spill):
        ctrl_sp, spd_sp = _np_reference(inputs, spill)
        control[spill] = ctrl_sp
        speed_pred[spill] = spd_sp

    return control, speed_pred


# revision 22
# speedup vs baseline: 1.3763x; 1.0098x over previous
"""Trainium2 Bass kernel for nn_CILRSModel (moe_routing).

Strategy:
  - Host-side MoE routing: rows are bucketed by `command` (6 branches) and
    distributed evenly over 8 cores. Each core gets a fixed [6 x CAP] row
    layout so the SPMD kernel statically knows which branch weights apply
    to which batch tile (no on-device routing control flow).
  - Host-side transpose: the embedding is shipped feature-major (partition =
    feature), per batch-tile contiguous, so every matmul operand already has
    the contraction dim on SBUF partitions and every x load is one fast DMA.
    The speed scalar is shipped broadcast across all 128 partitions so the
    speed-MLP's first (K=1) layer runs on the Scalar engine.
  - On device everything is feature-major fp16 (PSUM accumulates fp32).
    Two-stage software pipeline: tile t+1's first-layer matmuls are emitted
    before tile t's second/third layers so eviction latency hides behind
    independent PE work.
  - The two tiny-M output matmuls (control M=3, speed M=1) are packed into
    distinct PE column groups via tile_position so they run concurrently.
  - Outputs return feature-major as out4 = [4, rows] and are scattered back
    to the original row order on host.
"""

import os
import sys

import numpy as np

_TRN_REPO = "/opt/trn_rl_repo"
if _TRN_REPO not in sys.path:
    sys.path.insert(0, _TRN_REPO)

# Problem constants (hardcoded per harness contract)
B = 65536
D_EMB = 512
D_LAT = 128
H = 256
NBRANCH = 6
D_IN = D_EMB + D_LAT  # 640
NCORES = 8
CAP = 1408            # per-core per-branch row capacity (actual max ~1389)
BPC = NBRANCH * CAP   # 8448 rows per core
SLOT_TILES = [(0, 512), (512, 512), (1024, 384)]
WB_K = 1798           # per-branch weight-blob width: 5*256 + 2*256 + 2*3

# tile table: (branch k, col offset, width, xt element offset)
TILES = []
_eoff = 0
for _k in range(NBRANCH):
    for _off, _nt in SLOT_TILES:
        TILES.append((_k, _off, _nt, _eoff))
        _eoff += 128 * 5 * _nt
XT_ELEMS = _eoff

_cache = {}


# --------------------------------------------------------------------------
# Device kernel
# --------------------------------------------------------------------------

def _build_nc():
    if "nc" in _cache:
        return _cache["nc"]

    import concourse.mybir as mybir
    import concourse.tile as tile
    from concourse import bacc

    f32 = mybir.dt.float32
    f16 = getattr(mybir.dt, os.environ.get("KERNEL_DT", "float16"))
    AF = mybir.ActivationFunctionType
    ALU = mybir.AluOpType

    nc = bacc.Bacc("TRN2", target_bir_lowering=False, debug=False,
                   num_devices=NCORES)

    xt = nc.dram_tensor("xt", [XT_ELEMS], f16, kind="ExternalInput")[:]
    wa = nc.dram_tensor("wa", [128, 2050], f16, kind="ExternalInput")[:]
    wb = nc.dram_tensor("wb", [128, NBRANCH * WB_K], f16, kind="ExternalInput")[:]
    wc = nc.dram_tensor("wc", [128, 40], f32, kind="ExternalInput")[:]
    out4 = nc.dram_tensor("out4", [4, BPC], f32, kind="ExternalOutput")[:]

    with tile.TileContext(nc) as tc:
        with (
            tc.tile_pool(name="wpool", bufs=1) as wpool,
            tc.tile_pool(name="xpool", bufs=4) as xpool,
            tc.tile_pool(name="hpool", bufs=2) as hpool,
            tc.tile_pool(name="opool", bufs=1) as opool,
            tc.tile_pool(name="pps", bufs=1, space="PSUM") as pps,
        ):
            wc_s = wpool.tile([128, 40], f32, tag="wc")
            nc.sync.dma_start(out=wc_s[:], in_=wc)
            wa_s = wpool.tile([128, 2050], f16, tag="wa")
            nc.sync.dma_start(out=wa_s[:], in_=wa)
            wb_s = wpool.tile([128, NBRANCH * WB_K], f16, tag="wb")
            # branch 0 weights up front; later branches prefetched in-loop
            nc.sync.dma_start(out=wb_s[:, 0:WB_K], in_=wb[:, 0:WB_K])

            # f16 blob A slices
            def wsi2_sl(o):
                return wa_s[:, o * 128:(o + 1) * 128]

            def wso1_sl(o, j):
                c = 256 + o * 256 + j * 128
                return wa_s[:, c:c + 128]

            def wso2_sl(o, j):
                c = 1536 + o * 256 + j * 128
                return wa_s[:, c:c + 128]

            def wso3_sl(o):
                return wa_s[:, 2048 + o:2049 + o]

            # f16 blob B slices (per-branch-contiguous)
            def wb1_sl(o, k, j):
                c = k * WB_K + o * 256 + j * 128
                return wb_s[:, c:c + 128]

            def wb2_sl(o, k, j):
                c = k * WB_K + 1280 + o * 256 + j * 128
                return wb_s[:, c:c + 128]

            def wb3_sl(o, k):
                c = k * WB_K + 1792 + o * 3
                return wb_s[:, c:c + 3]

            # f32 blob slices (per-partition bias/scale columns)
            def bsi1_c(j):
                return wc_s[:, j:j + 1]

            bsi2_c = wc_s[:, 2:3]

            def bso1_c(j):
                return wc_s[:, 3 + j:4 + j]

            def bso2_c(j):
                return wc_s[:, 5 + j:6 + j]

            def wsi1_c(j):
                return wc_s[:, 7 + j:8 + j]

            def bb1_c(j, k):
                c = 9 + j * 6 + k
                return wc_s[:, c:c + 1]

            def bb2_c(j, k):
                c = 21 + j * 6 + k
                return wc_s[:, c:c + 1]

            bso3_c = wc_s[0:1, 33:34]
            bb3_c = wc_s[32:35, 34:40]   # bb3 lives at partitions 32:35

            ctl_s = opool.tile([35, BPC], f32, tag="octl")  # rows 32:35 used
            spd_s = opool.tile([1, BPC], f32, tag="ospd")

            # warm-up: pull the ACT table load forward (gated only on the tiny
            # wc blob) and keep the PE busy/HAM-warm before real tiles arrive
            warm = wpool.tile([1, 8], f32, tag="warm")
            nc.scalar.activation(warm[:], wc_s[0:1, 0:8], AF.Relu,
                                 bias=bsi1_c(0)[0:1])
            pwarm = pps.tile([128, 2, 512], f32, tag="pp2")
            for _ in range(8):
                nc.tensor.matmul(pwarm[:, 0, :], wa_s[:, 0:128], wa_s[:, 512:1024],
                                 start=True, stop=True)

            def evict_relu(dst, src, bias_ap, on_act):
                if on_act:
                    nc.scalar.activation(dst, src, AF.Relu, bias=bias_ap)
                else:
                    nc.vector.tensor_scalar(dst, src, bias_ap, 0.0, ALU.add, ALU.max)

            def sp_load(t):
                """x tile DMA + branch-weight prefetch."""
                k, off, nt, eoff = TILES[t]
                x_s = xpool.tile([128, 5, nt], f16, tag=f"x{nt}")
                src = xt[eoff:eoff + 128 * 5 * nt].rearrange(
                    "(p o b) -> p o b", p=128, o=5)
                nc.sync.dma_start(out=x_s[:], in_=src)
                if off == 0 and k + 1 < NBRANCH:
                    c = (k + 1) * WB_K
                    nc.sync.dma_start(out=wb_s[:, c:c + WB_K],
                                      in_=wb[:, c:c + WB_K])
                return (x_s, nt)

            def sp_act(t, ld):
                """speed-MLP layer 1 on ScalarE (scale+bias+relu)."""
                x_s, nt = ld
                hsp = hpool.tile([128, 2, nt], f16, tag=f"hsp{nt}")
                for j in range(2):
                    nc.scalar.activation(hsp[:, j, :], x_s[:, 4, :], AF.Relu,
                                         bias=bsi1_c(j), scale=wsi1_c(j))
                return (x_s, hsp, nt)

            def sp_mm(st, q1_tile):
                """speed-MLP layer 2 -> latent overwrites x_s[:,4,:].
                PSUM target aliases the (already-evicted) j1 bank of the
                current cycle's q1 tile - no extra bank allocation."""
                x_s, hsp, nt = st
                p_sp = q1_tile[:, 1, :]
                nc.tensor.matmul(p_sp[:, :nt], wsi2_sl(0), hsp[:, 0, :],
                                 start=True, stop=False)
                nc.tensor.matmul(p_sp[:, :nt], wsi2_sl(1), hsp[:, 1, :],
                                 start=False, stop=True)
                nc.vector.tensor_scalar(x_s[:, 4, :], p_sp[:, :nt], bsi2_c,
                                        None, ALU.add)

            def layer1(t, st):
                """Layer-1 of both heads + h1/g1 evictions."""
                x_s, hsp, nt = st
                k, off, _, _ = TILES[t]

                p1 = pps.tile([128, 2, 512], f32, tag="pp1")
                for j in range(2):
                    for o in range(4):
                        nc.tensor.matmul(p1[:, j, :nt], wso1_sl(o, j),
                                         x_s[:, o, :],
                                         start=(o == 0), stop=False)
                q1 = pps.tile([128, 2, 512], f32, tag="pq1")
                for j in range(2):
                    for o in range(4):
                        nc.tensor.matmul(q1[:, j, :nt], wb1_sl(o, k, j),
                                         x_s[:, o, :],
                                         start=(o == 0), stop=False)
                # latent subtile finishers (x_s[:,4,:] written last cycle)
                for j in range(2):
                    nc.tensor.matmul(p1[:, j, :nt], wso1_sl(4, j),
                                     x_s[:, 4, :], start=False, stop=True)
                for j in range(2):
                    nc.tensor.matmul(q1[:, j, :nt], wb1_sl(4, k, j),
                                     x_s[:, 4, :], start=False, stop=True)

                h1 = hpool.tile([128, 2, nt], f16, tag=f"h1{nt}")
                evict_relu(h1[:, 0, :], p1[:, 0, :nt], bso1_c(0), True)
                evict_relu(h1[:, 1, :], p1[:, 1, :nt], bso1_c(1), False)
                g1 = hpool.tile([128, 2, nt], f16, tag=f"g1{nt}")
                evict_relu(g1[:, 0, :], q1[:, 0, :nt], bb1_c(0, k), True)
                evict_relu(g1[:, 1, :], q1[:, 1, :nt], bb1_c(1, k), False)
                return (k, off, nt, h1, g1, q1)

            def l23_mm(st):
                """Layer-2 matmuls + h2/g2 evictions."""
                k, off, nt, h1, g1, _ = st
                p2 = pps.tile([128, 2, 512], f32, tag="pp2")
                q2 = pps.tile([128, 2, 512], f32, tag="pq2")
                # o-major order: the j1-eviction-dependent (o=1) matmuls run
                # last, giving the Vector engine extra slack
                for o in range(2):
                    for j in range(2):
                        nc.tensor.matmul(p2[:, j, :nt], wso2_sl(o, j),
                                         h1[:, o, :],
                                         start=(o == 0), stop=(o == 1))
                for o in range(2):
                    for j in range(2):
                        nc.tensor.matmul(q2[:, j, :nt], wb2_sl(o, k, j),
                                         g1[:, o, :],
                                         start=(o == 0), stop=(o == 1))

                h2 = hpool.tile([128, 2, nt], f16, tag=f"h2{nt}")
                evict_relu(h2[:, 0, :], p2[:, 0, :nt], bso2_c(0), True)
                evict_relu(h2[:, 1, :], p2[:, 1, :nt], bso2_c(1), False)
                g2 = hpool.tile([128, 2, nt], f16, tag=f"g2{nt}")
                evict_relu(g2[:, 0, :], q2[:, 0, :nt], bb2_c(0, k), True)
                evict_relu(g2[:, 1, :], q2[:, 1, :nt], bb2_c(1, k), False)
                return (k, off, nt, h2, g2, q2)

            def l23_tail(st):
                """Output layer into q2's (evicted) j0 bank + final evicts."""
                k, off, nt, h2, g2, q2 = st
                c0 = k * CAP + off
                cols = slice(c0, c0 + nt)

                # spd (M=1) in col group 0, ctrl (M=3) in col group 1
                nc.tensor.matmul(q2[0:1, 0, :nt], wso3_sl(0), h2[:, 0, :],
                                 start=True, stop=False, tile_position=(0, 0))
                nc.tensor.matmul(q2[0:1, 0, :nt], wso3_sl(1), h2[:, 1, :],
                                 start=False, stop=True, tile_position=(0, 0))
                nc.tensor.matmul(q2[32:35, 0, :nt], wb3_sl(0, k), g2[:, 0, :],
                                 start=True, stop=False, tile_position=(0, 32))
                nc.tensor.matmul(q2[32:35, 0, :nt], wb3_sl(1, k), g2[:, 1, :],
                                 start=False, stop=True, tile_position=(0, 32))
                nc.vector.tensor_scalar(spd_s[:, cols], q2[0:1, 0, :nt],
                                        bso3_c, None, ALU.add)
                nc.scalar.activation(ctl_s[32:35, cols], q2[32:35, 0, :nt],
                                     AF.Sigmoid, bias=bb3_c[:, k:k + 1])
                # stream results out once a branch slot's last tile is done
                if off == 1024:
                    kcols = slice(k * CAP, (k + 1) * CAP)
                    nc.sync.dma_start(out=out4[0:3, kcols],
                                      in_=ctl_s[32:35, kcols])
                    nc.sync.dma_start(out=out4[3:4, kcols],
                                      in_=spd_s[:, kcols])

            # pipeline: load(t+1) | l23_mm(t-2) | layer1(t-1) | tail(t-2)
            #           | sp_act(t+1) | sp_mm(t)
            NT_ = len(TILES)
            loads = {0: sp_load(0)}
            sp_states = {0: sp_act(0, loads.pop(0))}
            l1_states = {}
            l23_states = {}
            for t in range(NT_ + 2):
                if t + 1 < NT_:
                    loads[t + 1] = sp_load(t + 1)
                if t - 2 >= 0:
                    l23_states[t - 2] = l23_mm(l1_states.pop(t - 2))
                if 0 <= t - 1 < NT_:
                    l1_states[t - 1] = layer1(t - 1, sp_states.pop(t - 1))
                if t - 2 >= 0:
                    l23_tail(l23_states.pop(t - 2))
                if 0 < t + 1 < NT_:
                    sp_states[t + 1] = sp_act(t + 1, loads.pop(t + 1))
                if t < NT_:
                    if t >= 1:
                        q1_t = l1_states[t - 1][5]
                    else:
                        q1_t = pps.tile([128, 2, 512], f32, tag="pq1")
                    sp_mm(sp_states[t], q1_t)

    nc.compile()
    _cache["nc"] = nc
    return nc


# --------------------------------------------------------------------------
# Host-side routing / layout
# --------------------------------------------------------------------------

def _np16():
    if os.environ.get("KERNEL_DT", "float16") == "bfloat16":
        import ml_dtypes
        return ml_dtypes.bfloat16
    return np.float16


def _fm(w, dtype):
    """[K, ...] -> [128, K//128, ...] with contraction index f = o*128 + p."""
    ko = w.shape[0] // 128
    perm = (1, 0) + tuple(range(2, w.ndim + 1))
    return np.ascontiguousarray(
        w.reshape(ko, 128, *w.shape[1:]).transpose(*perm), dtype=dtype)


def _prep_weights(i):
    f32 = np.float32
    f16 = _np16()

    def a(x):
        return np.asarray(x, dtype=f32)

    # f16 blob A: [wsi2 | wso1 | wso2 | wso3] along free dim
    wsi2 = _fm(a(i["Wsi2"]), f16).reshape(128, -1)            # 256
    wso1 = _fm(a(i["Wso1"]), f16).reshape(128, -1)            # 1280
    wso2 = _fm(a(i["Wso2"]), f16).reshape(128, -1)            # 512
    wso3 = _fm(a(i["Wso3"]), f16).reshape(128, -1)            # 2
    wa = np.concatenate([wsi2, wso1, wso2, wso3], axis=1)
    assert wa.shape == (128, 2050)

    # f16 blob B, per-branch-contiguous: for each k [wb1_k | wb2_k | wb3_k]
    wb1 = _fm(a(i["Wb1"]).transpose(1, 0, 2), f16)   # [128, 5, 6, 256]
    wb2 = _fm(a(i["Wb2"]).transpose(1, 0, 2), f16)   # [128, 2, 6, 256]
    wb3 = _fm(a(i["Wb3"]).transpose(1, 0, 2), f16)   # [128, 2, 6, 3]
    parts = []
    for k in range(NBRANCH):
        parts.append(wb1[:, :, k, :].reshape(128, -1))   # 1280
        parts.append(wb2[:, :, k, :].reshape(128, -1))   # 512
        parts.append(wb3[:, :, k, :].reshape(128, -1))   # 6
    wb = np.ascontiguousarray(np.concatenate(parts, axis=1))
    assert wb.shape == (128, NBRANCH * WB_K), wb.shape

    # f32 blob: per-partition bias/scale columns
    wc = np.zeros((128, 40), dtype=f32)
    wc[:, 0:2] = _fm(a(i["bsi1"]), f32)
    wc[:, 2] = a(i["bsi2"])
    wc[:, 3:5] = _fm(a(i["bso1"]), f32)
    wc[:, 5:7] = _fm(a(i["bso2"]), f32)
    wc[:, 7:9] = _fm(a(i["Wsi1"]).reshape(256), f32)
    wc[:, 9:21] = _fm(a(i["bb1"]).T, f32).reshape(128, 12)
    wc[:, 21:33] = _fm(a(i["bb2"]).T, f32).reshape(128, 12)
    wc[0, 33] = a(i["bso3"])[0]
    wc[32:35, 34:40] = a(i["bb3"]).T
    return {"wa": wa, "wb": wb, "wc": wc}


def _route(cmd):
    """Assign rows to (core, slot-position); slot k of every core holds only
    branch-k rows. Returns idx [NCORES, BPC], valid [NCORES, BPC], spill."""
    idx = np.zeros((NCORES, BPC), dtype=np.int64)
    valid = np.zeros((NCORES, BPC), dtype=bool)
    spill = []
    for k in range(NBRANCH):
        rows = np.flatnonzero(cmd == k)
        for c, part in enumerate(np.array_split(rows, NCORES)):
            if len(part) > CAP:
                spill.append(part[CAP:])
                part = part[:CAP]
            idx[c, k * CAP:k * CAP + len(part)] = part
            valid[c, k * CAP:k * CAP + len(part)] = True
    spill = np.concatenate(spill) if spill else np.zeros(0, dtype=np.int64)
    return idx, valid, spill


def _np_reference(i, rows):
    """Exact reference math in numpy for a subset of rows (spill fallback)."""
    f32 = np.float32
    E = np.asarray(i["embedding"], f32)[rows]
    S = np.asarray(i["speed"], f32)[rows]
    cmd = np.asarray(i["command"])[rows].astype(np.int64) - 1
    sp = np.maximum(S @ np.asarray(i["Wsi1"], f32) + np.asarray(i["bsi1"], f32), 0)
    sp = sp @ np.asarray(i["Wsi2"], f32) + np.asarray(i["bsi2"], f32)
    emb = np.concatenate([E, sp], axis=1)
    hs = np.maximum(emb @ np.asarray(i["Wso1"], f32) + np.asarray(i["bso1"], f32), 0)
    hs = np.maximum(hs @ np.asarray(i["Wso2"], f32) + np.asarray(i["bso2"], f32), 0)
    spd = hs @ np.asarray(i["Wso3"], f32) + np.asarray(i["bso3"], f32)
    ctrl = np.zeros((len(rows), 3), f32)
    for k in range(NBRANCH):
        m = cmd == k
        if not m.any():
            continue
        h = np.maximum(emb[m] @ np.asarray(i["Wb1"], f32)[k]
                       + np.asarray(i["bb1"], f32)[k], 0)
        h = np.maximum(h @ np.asarray(i["Wb2"], f32)[k]
                       + np.asarray(i["bb2"], f32)[k], 0)
        z = h @ np.asarray(i["Wb3"], f32)[k] + np.asarray(i["bb3"], f32)[k]
        ctrl[m] = 1.0 / (1.0 + np.exp(-z))
    return ctrl, spd.astype(f32)


# --------------------------------------------------------------------------
# Entry point
# --------------------------------------------------------------------------

LAST_RESULTS = None  # BassKernelResults of the most recent device run


def kernel(embedding, speed, command, **weights):
    global LAST_RESULTS
    inputs = dict(weights)
    inputs.update(embedding=embedding, speed=speed, command=command)

    embedding = np.asarray(embedding, dtype=np.float32)
    speed = np.asarray(speed, dtype=np.float32)
    command_np = np.asarray(command)

    if embedding.shape != (B, D_EMB):
        # Unexpected problem size: fall back to exact host computation.
        ctrl, spd = _np_reference(inputs, np.arange(embedding.shape[0]))
        return ctrl, spd

    cmd = command_np.astype(np.int64) - 1
    idx, valid, spill = _route(cmd)

    w = _prep_weights(inputs)
    f16 = _np16()

    in_maps = []
    for c in range(NCORES):
        rows = idx[c]
        emb_t = embedding[rows].T.astype(f16)                  # [512, BPC]
        emb_t = emb_t.reshape(4, 128, BPC).transpose(1, 0, 2)  # [128, 4, BPC]
        spd_b = speed[rows, 0].astype(f16)                     # [BPC]
        xt = np.empty(XT_ELEMS, dtype=f16)
        for (k, off, nt, eoff) in TILES:
            c0 = k * CAP + off
            blk = xt[eoff:eoff + 128 * 5 * nt].reshape(128, 5, nt)
            blk[:, 0:4, :] = emb_t[:, :, c0:c0 + nt]
            blk[:, 4, :] = spd_b[c0:c0 + nt][None, :]
        m = {"xt": xt}
        m.update(w)
        in_maps.append(m)

    from concourse.bass_utils import run_bass_kernel_spmd

    nc = _build_nc()
    res = run_bass_kernel_spmd(
        nc, in_maps, core_ids=list(range(NCORES)),
        trace=bool(int(os.environ.get("KERNEL_TRACE", "0"))),
    )
    LAST_RESULTS = res

    control = np.zeros((B, 3), dtype=np.float32)
    speed_pred = np.zeros((B, 1), dtype=np.float32)
    for c in range(NCORES):
        o4 = np.asarray(res.results[c]["out4"])
        v = valid[c]
        rows = idx[c][v]
        control[rows] = o4[0:3, v].T
        speed_pred[rows, 0] = o4[3, v]

    if len(spill):
        ctrl_sp, spd_sp = _np_reference(inputs, spill)
        control[spill] = ctrl_sp
        speed_pred[spill] = spd_sp

    return control, speed_pred


# revision 24
# speedup vs baseline: 1.4033x; 1.0197x over previous
"""Trainium2 Bass kernel for nn_CILRSModel (moe_routing).

Strategy:
  - Host-side MoE routing: rows are bucketed by `command` (6 branches) and
    distributed evenly over 8 cores. Each core gets a fixed [6 x CAP] row
    layout so the SPMD kernel statically knows which branch weights apply
    to which batch tile (no on-device routing control flow).
  - Host-side transpose: the embedding is shipped feature-major (partition =
    feature), per batch-tile contiguous, so every matmul operand already has
    the contraction dim on SBUF partitions and every x load is one fast DMA.
    The speed scalar is shipped broadcast across all 128 partitions so the
    speed-MLP's first (K=1) layer runs on the Scalar engine.
  - On device everything is feature-major fp16 (PSUM accumulates fp32).
    Three-stage software pipeline - x-load/speed-MLP (t) | layer-1 (t-1) |
    layer-2+outputs (t-2) - so every matmul's dependencies are a full
    pipeline cycle old and the PE never stalls on evictions.
  - PSUM discipline: four single-buffer double-bank pools (p1/q1/p2/q2,
    8 banks total) whose slots recycle exactly one cycle apart; the
    speed-latent and output-layer matmuls alias already-evicted banks of
    q1/q2 instead of allocating their own.
  - The two tiny-M output matmuls (control M=3, speed M=1) are packed into
    distinct PE column groups via tile_position so they run concurrently.
  - A dummy activation (pulls the ACT table load forward) plus eight dummy
    matmuls absorb the NEFF preamble + HAM cold-clock ramp during startup.
  - Outputs return feature-major as out4 = [4, rows] and are scattered back
    to the original row order on host; branches that overflow the fixed
    per-core capacity (impossible for uniform commands) fall back to an
    exact host-side computation.
"""

import os
import sys

import numpy as np

_TRN_REPO = "/opt/trn_rl_repo"
if _TRN_REPO not in sys.path:
    sys.path.insert(0, _TRN_REPO)

# Problem constants (hardcoded per harness contract)
B = 65536
D_EMB = 512
D_LAT = 128
H = 256
NBRANCH = 6
D_IN = D_EMB + D_LAT  # 640
NCORES = 8
CAP = 1408            # per-core per-branch row capacity (actual max ~1389)
BPC = NBRANCH * CAP   # 8448 rows per core
SLOT_TILES = [(0, 512), (512, 512), (1024, 384)]
WB_K = 1798           # per-branch weight-blob width: 5*256 + 2*256 + 2*3

# tile table: (branch k, col offset, width, xt element offset)
TILES = []
_eoff = 0
for _k in range(NBRANCH):
    for _off, _nt in SLOT_TILES:
        TILES.append((_k, _off, _nt, _eoff))
        _eoff += 128 * 5 * _nt
XT_ELEMS = _eoff

_cache = {}


# --------------------------------------------------------------------------
# Device kernel
# --------------------------------------------------------------------------

def _build_nc():
    if "nc" in _cache:
        return _cache["nc"]

    import concourse.mybir as mybir
    import concourse.tile as tile
    from concourse import bacc

    f32 = mybir.dt.float32
    f16 = getattr(mybir.dt, os.environ.get("KERNEL_DT", "float16"))
    AF = mybir.ActivationFunctionType
    ALU = mybir.AluOpType

    nc = bacc.Bacc("TRN2", target_bir_lowering=False, debug=False,
                   num_devices=NCORES)

    xt = nc.dram_tensor("xt", [XT_ELEMS], f16, kind="ExternalInput")[:]
    wa = nc.dram_tensor("wa", [128, 2050], f16, kind="ExternalInput")[:]
    wb = nc.dram_tensor("wb", [128, NBRANCH * WB_K], f16, kind="ExternalInput")[:]
    wc = nc.dram_tensor("wc", [128, 40], f32, kind="ExternalInput")[:]
    out4 = nc.dram_tensor("out4", [4, BPC], f32, kind="ExternalOutput")[:]

    with tile.TileContext(nc) as tc:
        with (
            tc.tile_pool(name="wpool", bufs=1) as wpool,
            tc.tile_pool(name="xpool", bufs=5) as xpool,
            tc.tile_pool(name="hpool", bufs=3) as hpool,
            tc.tile_pool(name="opool", bufs=1) as opool,
            tc.tile_pool(name="pps", bufs=1, space="PSUM") as pps,
        ):
            wc_s = wpool.tile([128, 40], f32, tag="wc")
            nc.sync.dma_start(out=wc_s[:], in_=wc)
            wa_s = wpool.tile([128, 2050], f16, tag="wa")
            nc.sync.dma_start(out=wa_s[:], in_=wa)
            wb_s = wpool.tile([128, NBRANCH * WB_K], f16, tag="wb")

            # f16 blob A slices
            def wsi2_sl(o):
                return wa_s[:, o * 128:(o + 1) * 128]

            def wso1_sl(o, j):
                c = 256 + o * 256 + j * 128
                return wa_s[:, c:c + 128]

            def wso2_sl(o, j):
                c = 1536 + o * 256 + j * 128
                return wa_s[:, c:c + 128]

            def wso3_sl(o):
                return wa_s[:, 2048 + o:2049 + o]

            # f16 blob B slices (per-branch-contiguous)
            def wb1_sl(o, k, j):
                c = k * WB_K + o * 256 + j * 128
                return wb_s[:, c:c + 128]

            def wb2_sl(o, k, j):
                c = k * WB_K + 1280 + o * 256 + j * 128
                return wb_s[:, c:c + 128]

            def wb3_sl(o, k):
                c = k * WB_K + 1792 + o * 3
                return wb_s[:, c:c + 3]

            # f32 blob slices (per-partition bias/scale columns)
            def bsi1_c(j):
                return wc_s[:, j:j + 1]

            bsi2_c = wc_s[:, 2:3]

            def bso1_c(j):
                return wc_s[:, 3 + j:4 + j]

            def bso2_c(j):
                return wc_s[:, 5 + j:6 + j]

            def wsi1_c(j):
                return wc_s[:, 7 + j:8 + j]

            def bb1_c(j, k):
                c = 9 + j * 6 + k
                return wc_s[:, c:c + 1]

            def bb2_c(j, k):
                c = 21 + j * 6 + k
                return wc_s[:, c:c + 1]

            bso3_c = wc_s[0:1, 33:34]
            bb3_c = wc_s[32:35, 34:40]   # bb3 lives at partitions 32:35

            ctl_s = opool.tile([35, BPC], f32, tag="octl")  # rows 32:35 used
            spd_s = opool.tile([1, BPC], f32, tag="ospd")

            # warm-up: pull the ACT table load forward (gated only on the tiny
            # wc blob) and keep the PE busy/HAM-warm before real tiles arrive
            warm = wpool.tile([1, 8], f32, tag="warm")
            nc.scalar.activation(warm[:], wc_s[0:1, 0:8], AF.Relu,
                                 bias=bsi1_c(0)[0:1])
            pwarm = pps.tile([128, 2, 512], f32, tag="pp2")
            for _ in range(8):
                nc.tensor.matmul(pwarm[:, 0, :], wa_s[:, 0:128], wa_s[:, 512:1024],
                                 start=True, stop=True)

            def evict_relu(dst, src, bias_ap, on_act):
                if on_act:
                    nc.scalar.activation(dst, src, AF.Relu, bias=bias_ap)
                else:
                    nc.vector.tensor_scalar(dst, src, bias_ap, 0.0, ALU.add, ALU.max)

            def sp_load(t):
                """x tile DMA + branch-weight prefetch."""
                k, off, nt, eoff = TILES[t]
                x_s = xpool.tile([128, 5, nt], f16, tag=f"x{nt}")
                src = xt[eoff:eoff + 128 * 5 * nt].rearrange(
                    "(p o b) -> p o b", p=128, o=5)
                nc.sync.dma_start(out=x_s[:], in_=src)
                if off == 0 and k + 1 < NBRANCH:
                    c = (k + 1) * WB_K
                    nc.sync.dma_start(out=wb_s[:, c:c + WB_K],
                                      in_=wb[:, c:c + WB_K])
                return (x_s, nt)

            def sp_act(t, ld):
                """speed-MLP layer 1 on ScalarE (scale+bias+relu)."""
                x_s, nt = ld
                hsp = hpool.tile([128, 2, nt], f16, tag=f"hsp{nt}")
                for j in range(2):
                    nc.scalar.activation(hsp[:, j, :], x_s[:, 4, :], AF.Relu,
                                         bias=bsi1_c(j), scale=wsi1_c(j))
                return (x_s, hsp, nt)

            def sp_mm(st, q1_tile):
                """speed-MLP layer 2 -> latent overwrites x_s[:,4,:].
                PSUM target aliases the (already-evicted) j1 bank of the
                current cycle's q1 tile - no extra bank allocation."""
                x_s, hsp, nt = st
                p_sp = q1_tile[:, 1, :]
                nc.tensor.matmul(p_sp[:, :nt], wsi2_sl(0), hsp[:, 0, :],
                                 start=True, stop=False)
                nc.tensor.matmul(p_sp[:, :nt], wsi2_sl(1), hsp[:, 1, :],
                                 start=False, stop=True)
                nc.vector.tensor_scalar(x_s[:, 4, :], p_sp[:, :nt], bsi2_c,
                                        None, ALU.add)

            def layer1(t, st):
                """Layer-1 of both heads + h1/g1 evictions."""
                x_s, hsp, nt = st
                k, off, _, _ = TILES[t]

                p1 = pps.tile([128, 2, 512], f32, tag="pp1")
                for j in range(2):
                    for o in range(4):
                        nc.tensor.matmul(p1[:, j, :nt], wso1_sl(o, j),
                                         x_s[:, o, :],
                                         start=(o == 0), stop=False)
                q1 = pps.tile([128, 2, 512], f32, tag="pq1")
                for j in range(2):
                    for o in range(4):
                        nc.tensor.matmul(q1[:, j, :nt], wb1_sl(o, k, j),
                                         x_s[:, o, :],
                                         start=(o == 0), stop=False)
                # latent subtile finishers (x_s[:,4,:] written last cycle)
                for j in range(2):
                    nc.tensor.matmul(p1[:, j, :nt], wso1_sl(4, j),
                                     x_s[:, 4, :], start=False, stop=True)
                for j in range(2):
                    nc.tensor.matmul(q1[:, j, :nt], wb1_sl(4, k, j),
                                     x_s[:, 4, :], start=False, stop=True)

                h1 = hpool.tile([128, 2, nt], f16, tag=f"h1{nt}")
                evict_relu(h1[:, 0, :], p1[:, 0, :nt], bso1_c(0), True)
                evict_relu(h1[:, 1, :], p1[:, 1, :nt], bso1_c(1), False)
                g1 = hpool.tile([128, 2, nt], f16, tag=f"g1{nt}")
                evict_relu(g1[:, 0, :], q1[:, 0, :nt], bb1_c(0, k), True)
                evict_relu(g1[:, 1, :], q1[:, 1, :nt], bb1_c(1, k), False)
                return (k, off, nt, h1, g1, q1)

            def l23_mm(st):
                """Layer-2 matmuls + h2/g2 evictions."""
                k, off, nt, h1, g1, _ = st
                p2 = pps.tile([128, 2, 512], f32, tag="pp2")
                q2 = pps.tile([128, 2, 512], f32, tag="pq2")
                # o-major order: the j1-eviction-dependent (o=1) matmuls run
                # last, giving the Vector engine extra slack
                for o in range(2):
                    for j in range(2):
                        nc.tensor.matmul(p2[:, j, :nt], wso2_sl(o, j),
                                         h1[:, o, :],
                                         start=(o == 0), stop=(o == 1))
                for o in range(2):
                    for j in range(2):
                        nc.tensor.matmul(q2[:, j, :nt], wb2_sl(o, k, j),
                                         g1[:, o, :],
                                         start=(o == 0), stop=(o == 1))

                h2 = hpool.tile([128, 2, nt], f16, tag=f"h2{nt}")
                evict_relu(h2[:, 0, :], p2[:, 0, :nt], bso2_c(0), True)
                evict_relu(h2[:, 1, :], p2[:, 1, :nt], bso2_c(1), False)
                g2 = hpool.tile([128, 2, nt], f16, tag=f"g2{nt}")
                evict_relu(g2[:, 0, :], q2[:, 0, :nt], bb2_c(0, k), True)
                evict_relu(g2[:, 1, :], q2[:, 1, :nt], bb2_c(1, k), False)
                return (k, off, nt, h2, g2, q2)

            def l23_tail(st):
                """Output layer into q2's (evicted) j0 bank + final evicts."""
                k, off, nt, h2, g2, q2 = st
                c0 = k * CAP + off
                cols = slice(c0, c0 + nt)

                # spd (M=1) in col group 0, ctrl (M=3) in col group 1
                nc.tensor.matmul(q2[0:1, 0, :nt], wso3_sl(0), h2[:, 0, :],
                                 start=True, stop=False, tile_position=(0, 0))
                nc.tensor.matmul(q2[0:1, 0, :nt], wso3_sl(1), h2[:, 1, :],
                                 start=False, stop=True, tile_position=(0, 0))
                nc.tensor.matmul(q2[32:35, 0, :nt], wb3_sl(0, k), g2[:, 0, :],
                                 start=True, stop=False, tile_position=(0, 32))
                nc.tensor.matmul(q2[32:35, 0, :nt], wb3_sl(1, k), g2[:, 1, :],
                                 start=False, stop=True, tile_position=(0, 32))
                nc.vector.tensor_scalar(spd_s[:, cols], q2[0:1, 0, :nt],
                                        bso3_c, None, ALU.add)
                nc.scalar.activation(ctl_s[32:35, cols], q2[32:35, 0, :nt],
                                     AF.Sigmoid, bias=bb3_c[:, k:k + 1])
                # stream results out once a branch slot's last tile is done
                if off == 1024:
                    kcols = slice(k * CAP, (k + 1) * CAP)
                    nc.sync.dma_start(out=out4[0:3, kcols],
                                      in_=ctl_s[32:35, kcols])
                    nc.sync.dma_start(out=out4[3:4, kcols],
                                      in_=spd_s[:, kcols])

            # pipeline: load(t+1) | l23_mm(t-2) | layer1(t-1) | tail(t-2)
            #           | sp_act(t+1) | sp_mm(t)
            NT_ = len(TILES)
            loads = {0: sp_load(0)}
            # branch-0 weights right behind the first x tile on the queue
            nc.sync.dma_start(out=wb_s[:, 0:WB_K], in_=wb[:, 0:WB_K])
            sp_states = {0: sp_act(0, loads.pop(0))}
            l1_states = {}
            l23_states = {}
            for t in range(NT_ + 2):
                if t + 1 < NT_:
                    loads[t + 1] = sp_load(t + 1)
                if t - 2 >= 0:
                    l23_states[t - 2] = l23_mm(l1_states.pop(t - 2))
                if 0 <= t - 1 < NT_:
                    l1_states[t - 1] = layer1(t - 1, sp_states.pop(t - 1))
                if t - 2 >= 0:
                    l23_tail(l23_states.pop(t - 2))
                if 0 < t + 1 < NT_:
                    sp_states[t + 1] = sp_act(t + 1, loads.pop(t + 1))
                if t < NT_:
                    if t >= 1:
                        q1_t = l1_states[t - 1][5]
                    else:
                        q1_t = pps.tile([128, 2, 512], f32, tag="pq1")
                    sp_mm(sp_states[t], q1_t)

    nc.compile()
    _cache["nc"] = nc
    return nc


# --------------------------------------------------------------------------
# Host-side routing / layout
# --------------------------------------------------------------------------

def _np16():
    if os.environ.get("KERNEL_DT", "float16") == "bfloat16":
        import ml_dtypes
        return ml_dtypes.bfloat16
    return np.float16


def _fm(w, dtype):
    """[K, ...] -> [128, K//128, ...] with contraction index f = o*128 + p."""
    ko = w.shape[0] // 128
    perm = (1, 0) + tuple(range(2, w.ndim + 1))
    return np.ascontiguousarray(
        w.reshape(ko, 128, *w.shape[1:]).transpose(*perm), dtype=dtype)


def _prep_weights(i):
    f32 = np.float32
    f16 = _np16()

    def a(x):
        return np.asarray(x, dtype=f32)

    # f16 blob A: [wsi2 | wso1 | wso2 | wso3] along free dim
    wsi2 = _fm(a(i["Wsi2"]), f16).reshape(128, -1)            # 256
    wso1 = _fm(a(i["Wso1"]), f16).reshape(128, -1)            # 1280
    wso2 = _fm(a(i["Wso2"]), f16).reshape(128, -1)            # 512
    wso3 = _fm(a(i["Wso3"]), f16).reshape(128, -1)            # 2
    wa = np.concatenate([wsi2, wso1, wso2, wso3], axis=1)
    assert wa.shape == (128, 2050)

    # f16 blob B, per-branch-contiguous: for each k [wb1_k | wb2_k | wb3_k]
    wb1 = _fm(a(i["Wb1"]).transpose(1, 0, 2), f16)   # [128, 5, 6, 256]
    wb2 = _fm(a(i["Wb2"]).transpose(1, 0, 2), f16)   # [128, 2, 6, 256]
    wb3 = _fm(a(i["Wb3"]).transpose(1, 0, 2), f16)   # [128, 2, 6, 3]
    parts = []
    for k in range(NBRANCH):
        parts.append(wb1[:, :, k, :].reshape(128, -1))   # 1280
        parts.append(wb2[:, :, k, :].reshape(128, -1))   # 512
        parts.append(wb3[:, :, k, :].reshape(128, -1))   # 6
    wb = np.ascontiguousarray(np.concatenate(parts, axis=1))
    assert wb.shape == (128, NBRANCH * WB_K), wb.shape

    # f32 blob: per-partition bias/scale columns
    wc = np.zeros((128, 40), dtype=f32)
    wc[:, 0:2] = _fm(a(i["bsi1"]), f32)
    wc[:, 2] = a(i["bsi2"])
    wc[:, 3:5] = _fm(a(i["bso1"]), f32)
    wc[:, 5:7] = _fm(a(i["bso2"]), f32)
    wc[:, 7:9] = _fm(a(i["Wsi1"]).reshape(256), f32)
    wc[:, 9:21] = _fm(a(i["bb1"]).T, f32).reshape(128, 12)
    wc[:, 21:33] = _fm(a(i["bb2"]).T, f32).reshape(128, 12)
    wc[0, 33] = a(i["bso3"])[0]
    wc[32:35, 34:40] = a(i["bb3"]).T
    return {"wa": wa, "wb": wb, "wc": wc}


def _route(cmd):
    """Assign rows to (core, slot-position); slot k of every core holds only
    branch-k rows. Returns idx [NCORES, BPC], valid [NCORES, BPC], spill."""
    idx = np.zeros((NCORES, BPC), dtype=np.int64)
    valid = np.zeros((NCORES, BPC), dtype=bool)
    spill = []
    for k in range(NBRANCH):
        rows = np.flatnonzero(cmd == k)
        for c, part in enumerate(np.array_split(rows, NCORES)):
            if len(part) > CAP:
                spill.append(part[CAP:])
                part = part[:CAP]
            idx[c, k * CAP:k * CAP + len(part)] = part
            valid[c, k * CAP:k * CAP + len(part)] = True
    spill = np.concatenate(spill) if spill else np.zeros(0, dtype=np.int64)
    return idx, valid, spill


def _np_reference(i, rows):
    """Exact reference math in numpy for a subset of rows (spill fallback)."""
    f32 = np.float32
    E = np.asarray(i["embedding"], f32)[rows]
    S = np.asarray(i["speed"], f32)[rows]
    cmd = np.asarray(i["command"])[rows].astype(np.int64) - 1
    sp = np.maximum(S @ np.asarray(i["Wsi1"], f32) + np.asarray(i["bsi1"], f32), 0)
    sp = sp @ np.asarray(i["Wsi2"], f32) + np.asarray(i["bsi2"], f32)
    emb = np.concatenate([E, sp], axis=1)
    hs = np.maximum(emb @ np.asarray(i["Wso1"], f32) + np.asarray(i["bso1"], f32), 0)
    hs = np.maximum(hs @ np.asarray(i["Wso2"], f32) + np.asarray(i["bso2"], f32), 0)
    spd = hs @ np.asarray(i["Wso3"], f32) + np.asarray(i["bso3"], f32)
    ctrl = np.zeros((len(rows), 3), f32)
    for k in range(NBRANCH):
        m = cmd == k
        if not m.any():
            continue
        h = np.maximum(emb[m] @ np.asarray(i["Wb1"], f32)[k]
                       + np.asarray(i["bb1"], f32)[k], 0)
        h = np.maximum(h @ np.asarray(i["Wb2"], f32)[k]
                       + np.asarray(i["bb2"], f32)[k], 0)
        z = h @ np.asarray(i["Wb3"], f32)[k] + np.asarray(i["bb3"], f32)[k]
        ctrl[m] = 1.0 / (1.0 + np.exp(-z))
    return ctrl, spd.astype(f32)


# --------------------------------------------------------------------------
# Entry point
# --------------------------------------------------------------------------

LAST_RESULTS = None  # BassKernelResults of the most recent device run


def kernel(embedding, speed, command, **weights):
    global LAST_RESULTS
    inputs = dict(weights)
    inputs.update(embedding=embedding, speed=speed, command=command)

    embedding = np.asarray(embedding, dtype=np.float32)
    speed = np.asarray(speed, dtype=np.float32)
    command_np = np.asarray(command)

    if embedding.shape != (B, D_EMB):
        # Unexpected problem size: fall back to exact host computation.
        ctrl, spd = _np_reference(inputs, np.arange(embedding.shape[0]))
        return ctrl, spd

    cmd = command_np.astype(np.int64) - 1
    idx, valid, spill = _route(cmd)

    w = _prep_weights(inputs)
    f16 = _np16()

    in_maps = []
    for c in range(NCORES):
        rows = idx[c]
        emb_t = embedding[rows].T.astype(f16)                  # [512, BPC]
        emb_t = emb_t.reshape(4, 128, BPC).transpose(1, 0, 2)  # [128, 4, BPC]
        spd_b = speed[rows, 0].astype(f16)                     # [BPC]
        xt = np.empty(XT_ELEMS, dtype=f16)
        for (k, off, nt, eoff) in TILES:
            c0 = k * CAP + off
            blk = xt[eoff:eoff + 128 * 5 * nt].reshape(128, 5, nt)
            blk[:, 0:4, :] = emb_t[:, :, c0:c0 + nt]
            blk[:, 4, :] = spd_b[c0:c0 + nt][None, :]
        m = {"xt": xt}
        m.update(w)
        in_maps.append(m)

    from concourse.bass_utils import run_bass_kernel_spmd

    nc = _build_nc()
    res = run_bass_kernel_spmd(
        nc, in_maps, core_ids=list(range(NCORES)),
        trace=bool(int(os.environ.get("KERNEL_TRACE", "0"))),
    )
    LAST_RESULTS = res

    control = np.zeros((B, 3), dtype=np.float32)
    speed_pred = np.zeros((B, 1), dtype=np.float32)
    for c in range(NCORES):
        o4 = np.asarray(res.results[c]["out4"])
        v = valid[c]
        rows = idx[c][v]
        control[rows] = o4[0:3, v].T
        speed_pred[rows, 0] = o4[3, v]

    if len(spill):
        ctrl_sp, spd_sp = _np_reference(inputs, spill)
        control[spill] = ctrl_sp
        speed_pred[spill] = spd_sp

    return control, speed_pred
